# revision 21
# baseline (speedup 1.0000x reference)
"""Trainium2 Bass kernel for nn_CSCLoss: multi-scale bilinear point-sampling
cosine-consistency loss.

loss = 1 - mean_{pairs,(b,n)} <normalize(sample(feat_i, p_bn)), normalize(sample(feat_j, p_bn))>

Sharding: data-parallel over batch — 32 images -> 8 cores x 4 images; the
host sums the 8 per-core partial sums and applies the 1 - total/count
epilogue (the all-reduce of the sharding hint, done on 8 scalars).

Per-core dataflow (HBM-bandwidth-bound, ~22 MB/core):
 - Each level streams in DRAM-FLAT layout: partition p holds a contiguous
   1/128 slice of the level = 8 consecutive (b, c) rows, i.e.
   b = p//32, channels c = 8*(p%32)..8*(p%32)+7 each of H*W pixels.
   Both halves of the free dim go on the two HWDGE rings -> every
   descriptor is a big contiguous run (64 KB for lvl0), so the stream
   runs at HBM line rate with no small-descriptor trickle.
 - ONE ap_gather per level pulls all 4 bilinear corners (k) for all 32
   points (s) and all 8 channel offsets (o): out col j = s*32 + o*4 + k,
   idx = y0*W + x0 + dk(k) + o*H*W  (image-local; the partition encodes
   the image).  ap_gather's per-16-partition-group index blocks give each
   image's two groups their own point indices.  num_elems = 32768 for
   lvl0 = exactly the int16/2^15 ap_gather limit.
 - Index and corner-weight rows are computed on partition 0 by DVE from
   `boxes`, bounced through DRAM, and replicated into the [128, *] SBUF
   tiles with 0-stride SWDGE DMAs.  All staging for all levels runs
   before the first gather so the GpSimd FIFO never blocks staging.
 - Per-point channel sums: V_i*V_j products on DVE, then a matmul with a
   static [128, 4] image-selector contracts partitions -> PSUM [4, (s,o)],
   DVE reduces o -> [4, 32] per-point sums.  Cosine epilogue on [4, 32],
   final cross-image sum via a [4,1]x[4,1] matmul, one [1,1] DMA out.
"""

import sys
from contextlib import ExitStack

import numpy as np

if "/opt/trn_rl_repo" not in sys.path:
    sys.path.insert(0, "/opt/trn_rl_repo")

B, N, C = 32, 32, 256
LEVELS = [(64, 64), (32, 32), (16, 16)]  # (H, W)
N_CORES = 8
BL = B // N_CORES          # images per core
NPTS = BL * N              # 128 points per core
PAIRS = [(0, 1), (0, 2), (1, 2)]
EPS = 1e-12

_CACHE = {}


def _build_program():
    from concourse import bacc, bass, mybir, tile, library_config

    dt = mybir.dt
    AL = mybir.AluOpType

    nc = bacc.Bacc("TRN2", target_bir_lowering=False, debug=False)

    feats = [
        nc.dram_tensor(f"feat{i}", [BL, C, H, W], dt.float32, kind="ExternalInput")
        for i, (H, W) in enumerate(LEVELS)
    ]
    boxes = nc.dram_tensor("boxes", [BL, N, 4], dt.float32, kind="ExternalInput")
    out = nc.dram_tensor("out", [1, 1], dt.float32, kind="ExternalOutput")

    with tile.TileContext(nc) as tc, ExitStack() as ctx:
        pool = ctx.enter_context(tc.tile_pool(name="sbuf", bufs=1))
        pa = ctx.enter_context(tc.tile_pool(name="pa", bufs=1))
        pstream = ctx.enter_context(tc.tile_pool(name="stream", bufs=1))
        pwork = ctx.enter_context(tc.tile_pool(name="work", bufs=1))
        ppsum = ctx.enter_context(tc.tile_pool(name="psum", bufs=1, space="PSUM"))
        pdram = ctx.enter_context(tc.tile_pool(name="dram", bufs=1, space="DRAM"))

        nc.gpsimd.load_library(library_config.ap_gather)

        # ---- static setup ----
        # P4sel[p, m] = 1.0 iff p//32 == m  (colsum lhsT: contract partitions
        # into per-image rows);  P4T[m, p] = its transpose (replication lhsT).
        p4sel = pool.tile([128, 4], dt.float32)
        nc.vector.memset(p4sel[:], 0.0)
        for m in range(4):
            nc.vector.memset(p4sel[32 * m:32 * (m + 1), m:m + 1], 1.0)
        # p4t[m, col] = 1.0 iff col//32 == m, built partition-base-0 legal:
        # (col + 96*m)>>5 & 3 == (col//32 - m) mod 4 == 0  <=>  col//32 == m.
        p4i = pool.tile([4, 128], dt.int32, name="p4i")
        nc.gpsimd.iota(p4i[:], pattern=[[1, 128]], base=0, channel_multiplier=96)
        nc.vector.tensor_scalar(
            out=p4i[:], in0=p4i[:], scalar1=5, scalar2=3,
            op0=AL.arith_shift_right, op1=AL.bitwise_and,
        )
        p4t = pool.tile([4, 128], dt.float32)
        nc.vector.tensor_scalar(
            out=p4t[:], in0=p4i[:], scalar1=0, scalar2=None, op0=AL.is_equal,
        )
        ones4 = pool.tile([4, 1], dt.float32)
        nc.vector.memset(ones4[:], 1.0)

        # ---- boxes load first on the sync ring (phase A needs it) ----
        bxr = pool.tile([1, BL * N * 4], dt.float32)  # [1, 512] flat boxes
        nc.sync.dma_start(
            out=bxr[:].rearrange("o (a f) -> o a f", a=BL),
            in_=boxes.rearrange("b n c -> b (n c)"),
        )

        # ---- feature streams: DRAM-flat [128, E], halves on the 2 rings ----
        T_tiles = []
        for li, (H, W) in enumerate(LEVELS):
            E = BL * C * H * W // 128  # elems per partition (8 rows of H*W)
            fflat = feats[li].rearrange("b c h w -> (b c h w)").rearrange(
                "(p q) -> p q", p=128
            )
            T = pstream.tile([128, E], dt.float32, name=f"T{li}")
            nc.sync.dma_start(out=T[:, 0:E // 2], in_=fflat[:, 0:E // 2])
            nc.scalar.dma_start(out=T[:, E // 2:E], in_=fflat[:, E // 2:E])
            T_tiles.append(T)

        # ---- Phase A (per level): point math on partition 0 + staging.
        # All levels are square (W == H), so x and y coords are processed as
        # interleaved (cx, cy) PAIRS in one [1, 256]-wide op stream.
        # Per-level ordering (weights first, index rows last) keeps the
        # shared srow buffer free while the previous level's DRAM bounces
        # drain, so the three levels pipeline on DVE without stalls.
        bxv = bxr[:].rearrange("o (j c) -> o j c", c=4)
        coord2 = bxv[:, :, 0:2]  # [1, 128, 2] (cx, cy) pairs

        widxs, wbs = [], []
        for li, (H, W) in enumerate(LEVELS):
            HW = H * W
            E1 = float(W - 1)
            # pf = clip(coord*(E-1), 0, E-1); e0 = clamp(floor(pf), 0, E-2);
            # we = pf - e0.  floor via 16.16 fixed point (exact *2^16; the
            # <=2^-16 conversion slop is absorbed by the lerp weight).
            pf2 = pa.tile([1, 2 * NPTS], dt.float32, name="pf2", tag="pf2")
            pf2v = pf2[:].rearrange("o (s c) -> o s c", c=2)
            nc.vector.tensor_scalar(
                out=pf2v, in0=coord2, scalar1=E1, scalar2=0.0,
                op0=AL.mult, op1=AL.max,
            )
            nc.vector.tensor_scalar_min(out=pf2[:], in0=pf2[:], scalar1=E1)
            ifx2 = pa.tile([1, 2 * NPTS], dt.int32, name="ifx2", tag="ifx2")
            nc.vector.tensor_scalar(
                out=ifx2[:], in0=pf2[:], scalar1=65536.0, scalar2=None,
                op0=AL.mult,
            )
            nc.vector.tensor_scalar(
                out=ifx2[:], in0=ifx2[:], scalar1=16, scalar2=None,
                op0=AL.arith_shift_right,
            )
            e02 = pa.tile([1, 2 * NPTS], dt.float32, name="e02", tag="e02")
            nc.vector.tensor_scalar_min(out=e02[:], in0=ifx2[:], scalar1=float(W - 2))
            we2 = pa.tile([1, 2 * NPTS], dt.float32, name="we2", tag="we2")
            nc.vector.tensor_tensor(out=we2[:], in0=pf2[:], in1=e02[:], op=AL.subtract)
            w12 = pa.tile([1, 2 * NPTS], dt.float32, name="w12", tag="w12")
            nc.vector.tensor_scalar(
                out=w12[:], in0=we2[:], scalar1=-1.0, scalar2=1.0,
                op0=AL.mult, op1=AL.add,
            )
            e02v = e02[:].rearrange("o (s c) -> o s c", c=2)
            we2v = we2[:].rearrange("o (s c) -> o s c", c=2)
            w12v = w12[:].rearrange("o (s c) -> o s c", c=2)
            x0f, y0f = e02v[:, :, 0], e02v[:, :, 1]
            wx, wy = we2v[:, :, 0], we2v[:, :, 1]
            w1x, w1y = w12v[:, :, 0], w12v[:, :, 1]

            # corner weights w(b, s, k), k = yi*2 + xi  (k-major layout)
            wkt = pa.tile([1, 4 * NPTS], dt.float32, name="wkt", tag="wkt")
            for k, (wyt, wxt) in enumerate(
                [(w1y, w1x), (w1y, wx), (wy, w1x), (wy, wx)]
            ):
                nc.vector.tensor_tensor(
                    out=wkt[:, k * NPTS:(k + 1) * NPTS],
                    in0=wyt, in1=wxt, op=AL.mult,
                )
            # wrow[(b, s, k)] <- wkt[(k, b, s)], one strided copy
            wrow = pa.tile([1, NPTS * 4], dt.float32, name="wrow", tag="srow")
            nc.vector.tensor_copy(
                out=wrow[:].rearrange("o (b s k) -> o b s k", b=BL, k=4),
                in_=wkt[:].rearrange("o (k b s) -> o b s k", k=4, b=BL),
            )
            wsk = pdram.tile([BL, 128], dt.float32, name=f"wsk{li}")
            nc.gpsimd.dma_start(
                out=wsk[:], in_=wrow[:].rearrange("o (b c) -> o b c", b=BL),
            )
            # replicate each image's [1, (s k)] row to its 32 partitions via
            # the P4T matmul: wb[p, c] = wsk[p//32, c].
            s4f = pa.tile([BL, 128], dt.float32, name="s4f", tag="s4f")
            nc.gpsimd.dma_start(out=s4f[:], in_=wsk[:])
            wb_ps = ppsum.tile([128, 128], dt.float32, name=f"wbps{li}", tag="wbps")
            nc.tensor.matmul(wb_ps[:], p4t[:], s4f[:], start=True, stop=True)
            wb = pool.tile([128, 128], dt.float32, name=f"wb{li}")
            nc.vector.tensor_copy(out=wb[:], in_=wb_ps[:])
            wbs.append(wb)

            # basef[(b,s)] = y0*W + x0  (image-local: the partition holds b)
            basef = pa.tile([1, NPTS], dt.float32, name="basef", tag="basef")
            nc.vector.tensor_scalar(
                out=basef[:], in0=y0f, scalar1=float(W), scalar2=None,
                op0=AL.mult,
            )
            nc.vector.tensor_tensor(out=basef[:], in0=basef[:], in1=x0f, op=AL.add)

            # dk vector over the 32 (r, half) combos, laid out col = r*2+half:
            # k = r%4, o = half*4 + r//4, dk = (k//2)*W + k%2 + o*HW
            # (all of W, HW are powers of two -> pure shift/mask arithmetic)
            wlog = W.bit_length() - 1
            hwlog = HW.bit_length() - 1
            cvec = pa.tile([1, 32], dt.int32, name="cvec", tag="cvec")
            nc.gpsimd.iota(cvec[:], pattern=[[1, 32]], base=0, channel_multiplier=0)
            xk = pa.tile([1, 32], dt.int32, name="xk", tag="xk")
            nc.vector.tensor_scalar(
                out=xk[:], in0=cvec[:], scalar1=1, scalar2=1,
                op0=AL.arith_shift_right, op1=AL.bitwise_and,
            )
            ykw = pa.tile([1, 32], dt.int32, name="ykw", tag="ykw")
            nc.vector.tensor_scalar(
                out=ykw[:], in0=cvec[:], scalar1=2, scalar2=1,
                op0=AL.arith_shift_right, op1=AL.bitwise_and,
            )
            nc.vector.tensor_scalar(
                out=ykw[:], in0=ykw[:], scalar1=wlog, scalar2=None,
                op0=AL.logical_shift_left,
            )
            o4 = pa.tile([1, 32], dt.int32, name="o4", tag="o4")
            nc.vector.tensor_scalar(
                out=o4[:], in0=cvec[:], scalar1=1, scalar2=2,
                op0=AL.bitwise_and, op1=AL.logical_shift_left,
            )
            o2 = pa.tile([1, 32], dt.int32, name="o2", tag="o2")
            nc.vector.tensor_scalar(
                out=o2[:], in0=cvec[:], scalar1=3, scalar2=None,
                op0=AL.arith_shift_right,
            )
            nc.vector.tensor_tensor(out=o4[:], in0=o4[:], in1=o2[:], op=AL.add)
            nc.vector.tensor_scalar(
                out=o4[:], in0=o4[:], scalar1=hwlog, scalar2=None,
                op0=AL.logical_shift_left,
            )
            nc.vector.tensor_tensor(out=xk[:], in0=xk[:], in1=ykw[:], op=AL.add)
            nc.vector.tensor_tensor(out=xk[:], in0=xk[:], in1=o4[:], op=AL.add)
            dkf = pa.tile([1, 32], dt.float32, name="dkf", tag="dkf")
            nc.vector.tensor_copy(out=dkf[:], in_=xk[:])

            # wrapped index rows: gather-out col j = s*32 + o*4 + k reads the
            # group-local index at (partition r=j%16, col cb=j//16=s*2+half).
            # srow[1, 4096]: flat col = b*1024 + r*64 + s*2 + half;
            # value = basef[b,s] + dk(r,half).  Two broadcast adds (one per
            # half) build the whole row block.
            srow = pa.tile([1, 4 * 16 * 64], dt.float32, name="srow", tag="srow")
            srow_v = srow[:].rearrange(
                "o (b r s h) -> o b r s h", b=BL, r=16, s=32
            )
            basef_bc = (
                basef[:].rearrange("o (b s) -> o b s", b=BL)
                .unsqueeze(2).to_broadcast([1, BL, 16, 32])
            )
            dkf_v = dkf[:].rearrange("o (r h) -> o r h", h=2)
            for half in range(2):
                dk_bc = (
                    dkf_v[:, :, half].unsqueeze(1).unsqueeze(3)
                    .to_broadcast([1, BL, 16, 32])
                )
                nc.vector.tensor_tensor(
                    out=srow_v[:, :, :, :, half], in0=basef_bc, in1=dk_bc,
                    op=AL.add,
                )
            # write the wrapped rows to DRAM PRE-DUPLICATED ([128, 64]: each
            # image's [16, 64] block twice), then ONE plain contiguous-
            # partition read back.  (0-stride broadcast DMAs and multi-level
            # partition APs on the SBUF side both corrupt the transfer —
            # verified on HW.)
            sidx = pdram.tile([128, 64], dt.float32, name=f"sidx{li}")
            sidx_v = sidx[:].rearrange("(b d r) c -> b d r c", b=BL, d=2)
            srow_in = srow[:].rearrange("o (b r c) -> o b r c", b=BL, r=16)
            nc.gpsimd.dma_start(out=sidx_v[:, 0], in_=srow_in)
            nc.gpsimd.dma_start(out=sidx_v[:, 1], in_=srow_in)
            widx_f = pool.tile([128, 64], dt.float32, name=f"widxf{li}", tag="widxf")
            nc.gpsimd.dma_start(out=widx_f[:], in_=sidx[:])
            widx = pool.tile([128, 64], dt.int16, name=f"widx{li}")
            nc.vector.tensor_copy(out=widx[:], in_=widx_f[:])
            widxs.append(widx)

        # ---- gathers (one per level) + lerp ----
        V = [pool.tile([128, NPTS * 2], dt.float32, name=f"V{li}") for li in range(3)]
        for li, (H, W) in enumerate(LEVELS):
            HW = H * W
            E = BL * C * HW // 128
            og = pwork.tile([128, 1024], dt.float32, name=f"og{li}", tag="og")
            nc.gpsimd.ap_gather(
                out_ap=og[:], in_ap=T_tiles[li][:], idxs_ap=widxs[li][:],
                channels=128, num_elems=E, d=1, num_idxs=1024,
            )
            og_v = og[:].rearrange("c (s o k) -> c s o k", s=32, o=8)
            wb_bc = (
                wbs[li][:].rearrange("c (s k) -> c s k", s=32)
                .unsqueeze(2).to_broadcast([128, 32, 8, 4])
            )
            nc.vector.tensor_tensor(out=og_v, in0=og_v, in1=wb_bc, op=AL.mult)
            # sum the 4 corners -> V[p, s*8 + o]
            nc.vector.tensor_reduce(
                out=V[li][:],
                in_=og[:].rearrange("c (n k) -> c n k", k=4),
                axis=mybir.AxisListType.X, op=AL.add,
            )

        # ---- per-point channel sums: partitions contract via P4sel matmul.
        # prod[p, (s,o)] -> psum[4, (s,o)] -> reduce o -> [4, 32] (b, s).
        def colsum(name, vi, vj):
            prod = pwork.tile([128, NPTS * 2], dt.float32, name=f"prod{name}", tag="og")
            nc.vector.tensor_tensor(out=prod[:], in0=vi[:], in1=vj[:], op=AL.mult)
            ps = ppsum.tile([4, NPTS * 2], dt.float32, name=name)
            nc.tensor.matmul(ps[:], p4sel[:], prod[:], start=True, stop=True)
            sb = pool.tile([4, 32], dt.float32, name=f"sb{name}")
            nc.vector.tensor_reduce(
                out=sb[:], in_=ps[:].rearrange("p (s o) -> p s o", o=8),
                axis=mybir.AxisListType.X, op=AL.add,
            )
            return sb

        ss = [colsum(f"ss{li}", V[li], V[li]) for li in range(3)]
        dots = {(i, j): colsum(f"d{i}{j}", V[i], V[j]) for i, j in PAIRS}

        # ---- cosine epilogue on [4, 32] ----
        rns = []
        for li in range(3):
            nrm = pool.tile([4, 32], dt.float32, name=f"nrm{li}")
            nc.scalar.sqrt(out=nrm[:], in_=ss[li][:])
            nc.vector.tensor_scalar_max(out=nrm[:], in0=nrm[:], scalar1=EPS)
            rn = pool.tile([4, 32], dt.float32, name=f"rn{li}")
            nc.vector.reciprocal(out=rn[:], in_=nrm[:])
            rns.append(rn)

        tot = pool.tile([4, 32], dt.float32)
        first = True
        for i, j in PAIRS:
            t = pool.tile([4, 32], dt.float32, name=f"t{i}{j}")
            nc.vector.tensor_tensor(
                out=t[:], in0=dots[(i, j)][:], in1=rns[i][:], op=AL.mult
            )
            nc.vector.tensor_tensor(out=t[:], in0=t[:], in1=rns[j][:], op=AL.mult)
            if first:
                nc.vector.tensor_copy(out=tot[:], in_=t[:])
                first = False
            else:
                nc.vector.tensor_tensor(out=tot[:], in0=tot[:], in1=t[:], op=AL.add)

        tot4 = pool.tile([4, 1], dt.float32)
        nc.vector.tensor_reduce(
            out=tot4[:], in_=tot[:], axis=mybir.AxisListType.X, op=AL.add
        )
        res_ps = ppsum.tile([1, 1], dt.float32, name="resps")
        nc.tensor.matmul(res_ps[:], tot4[:], ones4[:], start=True, stop=True)
        res = pool.tile([1, 1], dt.float32)
        nc.vector.tensor_copy(out=res[:], in_=res_ps[:])
        nc.sync.dma_start(out=out.ap(), in_=res[:])

    nc.compile()
    return nc


def _get_program():
    if "nc" not in _CACHE:
        _CACHE["nc"] = _build_program()
    return _CACHE["nc"]


def _run_device(feat0, feat1, feat2, boxes, **run_kwargs):
    """Shard inputs batch-wise over the 8 cores, run the SPMD program, and
    return the BassKernelResults (one {"out": [1,1]} per core)."""
    from concourse.bass_utils import run_bass_kernel_spmd

    nc = _get_program()

    feats = [
        np.ascontiguousarray(np.asarray(f, dtype=np.float32))
        for f in (feat0, feat1, feat2)
    ]
    boxes = np.ascontiguousarray(np.asarray(boxes, dtype=np.float32))

    in_maps = []
    for k in range(N_CORES):
        sl = slice(k * BL, (k + 1) * BL)
        in_maps.append(
            {
                "feat0": feats[0][sl],
                "feat1": feats[1][sl],
                "feat2": feats[2][sl],
                "boxes": boxes[sl],
            }
        )

    return run_bass_kernel_spmd(
        nc, in_maps, core_ids=list(range(N_CORES)), **run_kwargs
    )


def kernel(feat0, feat1, feat2, boxes):
    r = _run_device(feat0, feat1, feat2, boxes)
    total = np.float64(0.0)
    for m in r.results:
        total += np.float64(m["out"].reshape(-1)[0])

    count = B * N * len(PAIRS)
    avg = np.float32(total) / np.float32(count)
    loss = np.float32(1.0) - avg
    loss = np.nan_to_num(loss, nan=0.0, posinf=1.0, neginf=0.0)
    return np.array(np.clip(loss, 0.0, 2.0), dtype=np.float32)


# revision 24
# speedup vs baseline: 1.1772x; 1.1772x over previous
"""Trainium2 Bass kernel for nn_CSCLoss: multi-scale bilinear point-sampling
cosine-consistency loss.

loss = 1 - mean_{pairs,(b,n)} <normalize(sample(feat_i, p_bn)), normalize(sample(feat_j, p_bn))>

Sharding: data-parallel over batch — 32 images -> 8 cores x 4 images; the
host sums the 8 per-core partial sums and applies the 1 - total/count
epilogue (the all-reduce of the sharding hint, done on 8 scalars).

Per-core dataflow (HBM-bandwidth-bound, ~22 MB/core):
 - Each level streams in DRAM-FLAT layout: partition p holds a contiguous
   1/128 slice of the level = 8 consecutive (b, c) rows, i.e. image
   b = p//32, channels c = 8*(p%32)..8*(p%32)+7, each of H*W pixels.
   The two halves of the free dim go on the two HWDGE rings -> every
   descriptor is a large contiguous run (64 KB for lvl0), so the stream
   runs at HBM line rate, big levels first.
 - ONE ap_gather per level pulls all 4 bilinear corners (k) for all 32
   points (s) and all 8 channel offsets (o): out col j = s*32 + o*4 + k,
   idx = y0*W + x0 + (k//2)*W + k%2 + o*H*W  (image-local; the partition
   encodes the image).  ap_gather's per-16-partition-group index blocks
   give each image's two groups their own points.  num_elems = 32768 for
   lvl0 = exactly the int16/2^15 ap_gather limit.
 - Staging has NO DMA round trips (SWDGE completion latency is 10-15 us
   under the feature stream): boxes load as [4, 128] (one partition per
   image), ALL point math runs on 4 partitions, and the [4, *] rows are
   replicated to [128, *] by matmuls with a static image-selector P4T.
   The wrapped-index column offset dk(p%16, half) is partition-local
   static iota arithmetic folded into one broadcast add.
 - Per-point channel sums: V_i*V_j products on DVE, then a matmul with
   the transposed selector contracts partitions -> PSUM [4, (s,o)], DVE
   reduces o -> [4, 32] per-point sums.  Cosine epilogue on [4, 32],
   final cross-image sum via a [4,1]x[4,1] matmul, one [1,1] DMA out.
"""

import sys
from contextlib import ExitStack

import numpy as np

if "/opt/trn_rl_repo" not in sys.path:
    sys.path.insert(0, "/opt/trn_rl_repo")

B, N, C = 32, 32, 256
LEVELS = [(64, 64), (32, 32), (16, 16)]  # (H, W), all square
N_CORES = 8
BL = B // N_CORES          # images per core
NPTS = BL * N              # 128 points per core
PAIRS = [(0, 1), (0, 2), (1, 2)]
EPS = 1e-12

_CACHE = {}


def _build_program():
    from concourse import bacc, bass, mybir, tile, library_config

    dt = mybir.dt
    AL = mybir.AluOpType

    nc = bacc.Bacc("TRN2", target_bir_lowering=False, debug=False)

    feats = [
        nc.dram_tensor(f"feat{i}", [BL, C, H, W], dt.float32, kind="ExternalInput")
        for i, (H, W) in enumerate(LEVELS)
    ]
    boxes = nc.dram_tensor("boxes", [BL, N, 4], dt.float32, kind="ExternalInput")
    out = nc.dram_tensor("out", [1, 1], dt.float32, kind="ExternalOutput")

    with tile.TileContext(nc) as tc, ExitStack() as ctx:
        pool = ctx.enter_context(tc.tile_pool(name="sbuf", bufs=1))
        pa = ctx.enter_context(tc.tile_pool(name="pa", bufs=1))
        pstream = ctx.enter_context(tc.tile_pool(name="stream", bufs=1))
        pwork = ctx.enter_context(tc.tile_pool(name="work", bufs=2))
        ppsum = ctx.enter_context(tc.tile_pool(name="psum", bufs=1, space="PSUM"))

        nc.gpsimd.load_library(library_config.ap_gather)

        # ---- static setup ----
        # P4sel[p, m] = 1.0 iff p//32 == m  (colsum lhsT: contract partitions
        # into per-image rows);  p4t[m, p] = its transpose (replication lhsT):
        # (col + 96*m)>>5 & 3 == (col//32 - m) mod 4 == 0  <=>  col//32 == m.
        p4sel = pool.tile([128, 4], dt.float32)
        nc.vector.memset(p4sel[:], 0.0)
        for m in range(4):
            nc.vector.memset(p4sel[32 * m:32 * (m + 1), m:m + 1], 1.0)
        p4i = pool.tile([4, 128], dt.int32, name="p4i")
        nc.gpsimd.iota(p4i[:], pattern=[[1, 128]], base=0, channel_multiplier=96)
        nc.vector.tensor_scalar(
            out=p4i[:], in0=p4i[:], scalar1=5, scalar2=3,
            op0=AL.arith_shift_right, op1=AL.bitwise_and,
        )
        p4t = pool.tile([4, 128], dt.float32)
        nc.vector.tensor_scalar(
            out=p4t[:], in0=p4i[:], scalar1=0, scalar2=None, op0=AL.is_equal,
        )
        ones4 = pool.tile([4, 1], dt.float32)
        nc.vector.memset(ones4[:], 1.0)
        # pid[p] = p for the static dk arithmetic
        pid = pool.tile([128, 1], dt.int32, name="pid")
        nc.gpsimd.iota(pid[:], pattern=[[1, 1]], base=0, channel_multiplier=1)

        # ---- boxes load first on the sync ring: one partition per image ----
        bx4 = pool.tile([BL, N * 4], dt.float32)
        nc.sync.dma_start(out=bx4[:], in_=boxes.rearrange("b n c -> b (n c)"))

        # ---- feature streams: DRAM-flat [128, E], halves on the 2 rings ----
        T_tiles = []
        for li, (H, W) in enumerate(LEVELS):
            E = BL * C * H * W // 128  # elems per partition (8 rows of H*W)
            fflat = feats[li].rearrange("b c h w -> (b c h w)").rearrange(
                "(p q) -> p q", p=128
            )
            T = pstream.tile([128, E], dt.float32, name=f"T{li}")
            nc.sync.dma_start(out=T[:, 0:E // 2], in_=fflat[:, 0:E // 2])
            nc.scalar.dma_start(out=T[:, E // 2:E], in_=fflat[:, E // 2:E])
            T_tiles.append(T)

        # ---- Phase A (per level): point math on 4 partitions (one per
        # image), replicated to 128 via P4T matmuls.  No DMA round trips.
        # All levels are square, so x/y process as interleaved (cx, cy)
        # pairs in one [4, 64]-wide op stream.
        bxv = bx4[:].rearrange("b (n c) -> b n c", c=4)
        coord2 = bxv[:, :, 0:2]  # [4, 32, 2] (cx, cy)

        widxs, wbs = [], []
        for li, (H, W) in enumerate(LEVELS):
            HW = H * W
            wlog = W.bit_length() - 1
            hwlog = HW.bit_length() - 1
            E1 = float(W - 1)

            # pf = clip(coord*(E-1), 0, E-1); e0 = clamp(floor(pf), 0, E-2);
            # we = pf - e0.  floor via 16.16 fixed point (exact *2^16; the
            # <=2^-16 conversion slop is absorbed by the lerp weight).
            pf2 = pa.tile([BL, 2 * N], dt.float32, name="pf2", tag="pf2")
            nc.vector.tensor_scalar(
                out=pf2[:].rearrange("b (s c) -> b s c", c=2), in0=coord2,
                scalar1=E1, scalar2=0.0, op0=AL.mult, op1=AL.max,
            )
            nc.vector.tensor_scalar_min(out=pf2[:], in0=pf2[:], scalar1=E1)
            ifx2 = pa.tile([BL, 2 * N], dt.int32, name="ifx2", tag="ifx2")
            nc.vector.tensor_scalar(
                out=ifx2[:], in0=pf2[:], scalar1=65536.0, scalar2=None,
                op0=AL.mult,
            )
            nc.vector.tensor_scalar(
                out=ifx2[:], in0=ifx2[:], scalar1=16, scalar2=None,
                op0=AL.arith_shift_right,
            )
            e02 = pa.tile([BL, 2 * N], dt.float32, name="e02", tag="e02")
            nc.vector.tensor_scalar_min(out=e02[:], in0=ifx2[:], scalar1=float(W - 2))
            we2 = pa.tile([BL, 2 * N], dt.float32, name="we2", tag="we2")
            nc.vector.tensor_tensor(out=we2[:], in0=pf2[:], in1=e02[:], op=AL.subtract)
            w12 = pa.tile([BL, 2 * N], dt.float32, name="w12", tag="w12")
            nc.vector.tensor_scalar(
                out=w12[:], in0=we2[:], scalar1=-1.0, scalar2=1.0,
                op0=AL.mult, op1=AL.add,
            )
            e02v = e02[:].rearrange("b (s c) -> b s c", c=2)
            we2v = we2[:].rearrange("b (s c) -> b s c", c=2)
            w12v = w12[:].rearrange("b (s c) -> b s c", c=2)
            x0f, y0f = e02v[:, :, 0], e02v[:, :, 1]
            wx, wy = we2v[:, :, 0], we2v[:, :, 1]
            w1x, w1y = w12v[:, :, 0], w12v[:, :, 1]

            # corner weight rows wrow4[b, s*4 + k] = wyk(b,s) * wxk(b,s)
            wrow4 = pa.tile([BL, 4 * N], dt.float32, name="wrow4", tag="wrow4")
            wrow4_v = wrow4[:].rearrange("b (s k) -> b s k", k=4)
            for k, (wyt, wxt) in enumerate(
                [(w1y, w1x), (w1y, wx), (wy, w1x), (wy, wx)]
            ):
                nc.vector.tensor_tensor(
                    out=wrow4_v[:, :, k], in0=wyt, in1=wxt, op=AL.mult,
                )
            # wb[p, (s k)] = wrow4[p//32, (s k)]
            wb_ps = ppsum.tile([128, 4 * N], dt.float32, name=f"wbps{li}", tag="wbps")
            nc.tensor.matmul(wb_ps[:], p4t[:], wrow4[:], start=True, stop=True)
            wb = pool.tile([128, 4 * N], dt.float32, name=f"wb{li}")
            nc.vector.tensor_copy(out=wb[:], in_=wb_ps[:])
            wbs.append(wb)

            # base4[b, s] = y0*W + x0  (image-local pixel index)
            base4 = pa.tile([BL, N], dt.float32, name="base4", tag="base4")
            nc.vector.tensor_scalar(
                out=base4[:], in0=y0f, scalar1=float(W), scalar2=None,
                op0=AL.mult,
            )
            nc.vector.tensor_tensor(out=base4[:], in0=base4[:], in1=x0f, op=AL.add)
            # basefP[p, s] = base4[p//32, s]
            bp_ps = ppsum.tile([128, N], dt.float32, name=f"bpps{li}", tag="bpps")
            nc.tensor.matmul(bp_ps[:], p4t[:], base4[:], start=True, stop=True)
            basefP = pa.tile([128, N], dt.float32, name="basefP", tag="basefP")
            nc.vector.tensor_copy(out=basefP[:], in_=bp_ps[:])

            # static per-partition wrapped-index offset: with r = p%16,
            # k = r%4, dkcol[p] = (k//2)*W + k%2 + (r//4)*HW; the second
            # index column (half = 1) adds 4*HW.
            dk2 = pa.tile([128, 2], dt.int32, name="dk2", tag="dk2")
            t1 = pa.tile([128, 1], dt.int32, name="t1", tag="t1")
            nc.vector.tensor_scalar(
                out=t1[:], in0=pid[:], scalar1=1, scalar2=None,
                op0=AL.bitwise_and,
            )  # xk = p & 1
            t2 = pa.tile([128, 1], dt.int32, name="t2", tag="t2")
            nc.vector.tensor_scalar(
                out=t2[:], in0=pid[:], scalar1=1, scalar2=1,
                op0=AL.arith_shift_right, op1=AL.bitwise_and,
            )  # yk = (p>>1) & 1
            nc.vector.tensor_scalar(
                out=t2[:], in0=t2[:], scalar1=wlog, scalar2=None,
                op0=AL.logical_shift_left,
            )  # yk * W
            nc.vector.tensor_tensor(out=t1[:], in0=t1[:], in1=t2[:], op=AL.add)
            nc.vector.tensor_scalar(
                out=t2[:], in0=pid[:], scalar1=2, scalar2=3,
                op0=AL.arith_shift_right, op1=AL.bitwise_and,
            )  # r//4 = (p>>2) & 3
            nc.vector.tensor_scalar(
                out=t2[:], in0=t2[:], scalar1=hwlog, scalar2=None,
                op0=AL.logical_shift_left,
            )  # (r//4) * HW
            nc.vector.tensor_tensor(out=t1[:], in0=t1[:], in1=t2[:], op=AL.add)
            nc.vector.tensor_scalar(
                out=dk2[:, 0:1], in0=t1[:], scalar1=0, scalar2=None, op0=AL.add,
            )
            nc.vector.tensor_scalar(
                out=dk2[:, 1:2], in0=t1[:], scalar1=4 * HW, scalar2=None,
                op0=AL.add,
            )
            dk2f = pa.tile([128, 2], dt.float32, name="dk2f", tag="dk2f")
            nc.vector.tensor_copy(out=dk2f[:], in_=dk2[:])

            # widx[p, s*2 + half] = basefP[p, s] + dk2f[p, half]
            # (gather-out col j = s*32 + o*4 + k reads the group-local index
            # at partition r = j%16, col cb = j//16 = s*2 + half, with
            # o = half*4 + r//4, k = r%4)
            widxf = pa.tile([128, 2 * N], dt.float32, name="widxf", tag="widxf")
            nc.vector.tensor_tensor(
                out=widxf[:].rearrange("p (s h) -> p s h", h=2),
                in0=basefP[:].unsqueeze(2).to_broadcast([128, N, 2]),
                in1=dk2f[:].unsqueeze(1).to_broadcast([128, N, 2]),
                op=AL.add,
            )
            widx = pool.tile([128, 2 * N], dt.int16, name=f"widx{li}")
            nc.vector.tensor_copy(out=widx[:], in_=widxf[:])
            widxs.append(widx)

        # ---- gathers (one per level) + lerp ----
        V = [pool.tile([128, NPTS * 2], dt.float32, name=f"V{li}") for li in range(3)]
        for li, (H, W) in enumerate(LEVELS):
            HW = H * W
            E = BL * C * HW // 128
            og = pwork.tile([128, 1024], dt.float32, name=f"og{li}", tag="og")
            nc.gpsimd.ap_gather(
                out_ap=og[:], in_ap=T_tiles[li][:], idxs_ap=widxs[li][:],
                channels=128, num_elems=E, d=1, num_idxs=1024,
            )
            og_v = og[:].rearrange("c (s o k) -> c s o k", s=32, o=8)
            wb_bc = (
                wbs[li][:].rearrange("c (s k) -> c s k", s=32)
                .unsqueeze(2).to_broadcast([128, 32, 8, 4])
            )
            nc.vector.tensor_tensor(out=og_v, in0=og_v, in1=wb_bc, op=AL.mult)
            # sum the 4 corners -> V[p, s*8 + o]
            nc.vector.tensor_reduce(
                out=V[li][:],
                in_=og[:].rearrange("c (n k) -> c n k", k=4),
                axis=mybir.AxisListType.X, op=AL.add,
            )

        # ---- per-point channel sums: partitions contract via P4sel matmul.
        # prod[p, (s,o)] -> psum[4, (s,o)] -> reduce o -> [4, 32] (b, s).
        def colsum(name, vi, vj):
            prod = pwork.tile([128, NPTS * 2], dt.float32, name=f"prod{name}", tag="og")
            nc.vector.tensor_tensor(out=prod[:], in0=vi[:], in1=vj[:], op=AL.mult)
            ps = ppsum.tile([4, NPTS * 2], dt.float32, name=name, tag="cs")
            nc.tensor.matmul(ps[:], p4sel[:], prod[:], start=True, stop=True)
            sb = pool.tile([4, 32], dt.float32, name=f"sb{name}")
            nc.vector.tensor_reduce(
                out=sb[:], in_=ps[:].rearrange("p (s o) -> p s o", o=8),
                axis=mybir.AxisListType.X, op=AL.add,
            )
            return sb

        ss = [colsum(f"ss{li}", V[li], V[li]) for li in range(3)]
        dots = {(i, j): colsum(f"d{i}{j}", V[i], V[j]) for i, j in PAIRS}

        # ---- cosine epilogue on [4, 32] ----
        rns = []
        for li in range(3):
            nrm = pool.tile([4, 32], dt.float32, name=f"nrm{li}")
            nc.scalar.sqrt(out=nrm[:], in_=ss[li][:])
            nc.vector.tensor_scalar_max(out=nrm[:], in0=nrm[:], scalar1=EPS)
            rn = pool.tile([4, 32], dt.float32, name=f"rn{li}")
            nc.vector.reciprocal(out=rn[:], in_=nrm[:])
            rns.append(rn)

        tot = pool.tile([4, 32], dt.float32)
        first = True
        for i, j in PAIRS:
            t = pool.tile([4, 32], dt.float32, name=f"t{i}{j}")
            nc.vector.tensor_tensor(
                out=t[:], in0=dots[(i, j)][:], in1=rns[i][:], op=AL.mult
            )
            nc.vector.tensor_tensor(out=t[:], in0=t[:], in1=rns[j][:], op=AL.mult)
            if first:
                nc.vector.tensor_copy(out=tot[:], in_=t[:])
                first = False
            else:
                nc.vector.tensor_tensor(out=tot[:], in0=tot[:], in1=t[:], op=AL.add)

        tot4 = pool.tile([4, 1], dt.float32)
        nc.vector.tensor_reduce(
            out=tot4[:], in_=tot[:], axis=mybir.AxisListType.X, op=AL.add
        )
        res_ps = ppsum.tile([1, 1], dt.float32, name="resps")
        nc.tensor.matmul(res_ps[:], tot4[:], ones4[:], start=True, stop=True)
        res = pool.tile([1, 1], dt.float32)
        nc.vector.tensor_copy(out=res[:], in_=res_ps[:])
        nc.sync.dma_start(out=out.ap(), in_=res[:])

    nc.compile()
    return nc


def _get_program():
    if "nc" not in _CACHE:
        _CACHE["nc"] = _build_program()
    return _CACHE["nc"]


def _run_device(feat0, feat1, feat2, boxes, **run_kwargs):
    """Shard inputs batch-wise over the 8 cores, run the SPMD program, and
    return the BassKernelResults (one {"out": [1,1]} per core)."""
    from concourse.bass_utils import run_bass_kernel_spmd

    nc = _get_program()

    feats = [
        np.ascontiguousarray(np.asarray(f, dtype=np.float32))
        for f in (feat0, feat1, feat2)
    ]
    boxes = np.ascontiguousarray(np.asarray(boxes, dtype=np.float32))

    in_maps = []
    for k in range(N_CORES):
        sl = slice(k * BL, (k + 1) * BL)
        in_maps.append(
            {
                "feat0": feats[0][sl],
                "feat1": feats[1][sl],
                "feat2": feats[2][sl],
                "boxes": boxes[sl],
            }
        )

    return run_bass_kernel_spmd(
        nc, in_maps, core_ids=list(range(N_CORES)), **run_kwargs
    )


def kernel(feat0, feat1, feat2, boxes):
    r = _run_device(feat0, feat1, feat2, boxes)
    total = np.float64(0.0)
    for m in r.results:
        total += np.float64(m["out"].reshape(-1)[0])

    count = B * N * len(PAIRS)
    avg = np.float32(total) / np.float32(count)
    loss = np.float32(1.0) - avg
    loss = np.nan_to_num(loss, nan=0.0, posinf=1.0, neginf=0.0)
    return np.array(np.clip(loss, 0.0, 2.0), dtype=np.float32)


# revision 27
# speedup vs baseline: 1.9610x; 1.6658x over previous
"""Trainium2 Bass kernel for nn_CSCLoss: multi-scale bilinear point-sampling
cosine-consistency loss.

loss = 1 - mean_{pairs,(b,n)} <normalize(sample(feat_i, p_bn)), normalize(sample(feat_j, p_bn))>

Sharding: data-parallel over batch — 32 images -> 8 cores x 4 images; the
host sums the 8 per-core partial sums and applies the 1 - total/count
epilogue (the all-reduce of the sharding hint, done on 8 scalars).

Key structural facts (HW-measured):
 - ap_gather costs ~30 Q7 cycles PER OUTPUT COLUMN regardless of
   batching; minimizing gather columns is the only lever.  Columns drop
   8x by gathering with d=8: the HOST pre-arranges each level as
   [128, H*W, 8] per core — partition p = (image b = p//32, chunk
   q = p%32) holds channels 8q..8q+7 CHANNEL-LAST, so one index fetches
   a corner's 8 channel values as one contiguous 32 B run.  128 columns
   per level (32 points x 4 corners), 384 total (~12 us on Pool).
 - The DMA streams the pre-arranged arrays as plain contiguous [128, E]
   tiles split across both HWDGE rings -> HBM line rate, big level
   first.  num_elems = H*W = 4096 (lvl0) meets ap_gather's
   num_elems*d*4/4 <= 2^15 limit exactly.
 - SWDGE DMA round trips cost 10-15 us under the feature stream, so
   staging uses NONE: boxes load as [16, 32] (partition (b, slo) holds
   the 8 points s = 4*s4+slo), point math runs on 16 partitions, and
   static iota-built selector matmuls (P16sel / P16b) replicate index
   bases and corner weights to the [128, *] gather layout.
 - Gather-out col j = s4*16 + slo*4 + k at partition group r = j%16
   (slo = r//4, k = r%4), cb = j//16 = s4: idx = y0*W + x0 + dk(k),
   image-local.  Weights w(b, s, k) broadcast over the d=8 channel dim.
 - Per-point channel sums: V_i*V_j products, P4sel matmul contracts
   partitions -> PSUM [4, (s, jj)], reduce jj -> [4, 32] per point.
   Cosine epilogue on [4, 32]; final cross-image sum via a [4,1]x[4,1]
   matmul; one [1,1] DMA out per core.
"""

import sys
from contextlib import ExitStack

import numpy as np

if "/opt/trn_rl_repo" not in sys.path:
    sys.path.insert(0, "/opt/trn_rl_repo")

B, N, C = 32, 32, 256
LEVELS = [(64, 64), (32, 32), (16, 16)]  # (H, W), all square
N_CORES = 8
BL = B // N_CORES          # images per core
NPTS = BL * N              # 128 points per core
PAIRS = [(0, 1), (0, 2), (1, 2)]
EPS = 1e-12

_CACHE = {}


def _build_program():
    from concourse import bacc, bass, mybir, tile, library_config

    dt = mybir.dt
    AL = mybir.AluOpType

    nc = bacc.Bacc("TRN2", target_bir_lowering=False, debug=False)

    feats = [
        nc.dram_tensor(
            f"feat{i}", [128, H * W * 8], dt.float32, kind="ExternalInput"
        )
        for i, (H, W) in enumerate(LEVELS)
    ]
    boxes = nc.dram_tensor("boxes", [BL, N, 4], dt.float32, kind="ExternalInput")
    out = nc.dram_tensor("out", [1, 1], dt.float32, kind="ExternalOutput")

    with tile.TileContext(nc) as tc, ExitStack() as ctx:
        pool = ctx.enter_context(tc.tile_pool(name="sbuf", bufs=1))
        pa = ctx.enter_context(tc.tile_pool(name="pa", bufs=1))
        pstream = ctx.enter_context(tc.tile_pool(name="stream", bufs=1))
        pwork = ctx.enter_context(tc.tile_pool(name="work", bufs=2))
        ppsum = ctx.enter_context(tc.tile_pool(name="psum", bufs=1, space="PSUM"))

        nc.gpsimd.load_library(library_config.ap_gather)

        # ---- static setup ----
        # P4sel[p, m] = 1.0 iff p//32 == m (colsum lhsT, 32-aligned memsets)
        p4sel = pool.tile([128, 4], dt.float32)
        nc.vector.memset(p4sel[:], 0.0)
        for m in range(4):
            nc.vector.memset(p4sel[32 * m:32 * (m + 1), m:m + 1], 1.0)
        ones4 = pool.tile([4, 1], dt.float32)
        nc.vector.memset(ones4[:], 1.0)
        # pid[p] = p
        pid = pool.tile([128, 1], dt.int32, name="pid")
        nc.gpsimd.iota(pid[:], pattern=[[1, 1]], base=0, channel_multiplier=1)
        # c1[kk, col] = col ; dif[kk, col] = kk  (on 16 partitions)
        c1 = pa.tile([16, 128], dt.int32, name="c1", tag="c1")
        nc.gpsimd.iota(c1[:], pattern=[[1, 128]], base=0, channel_multiplier=0)
        pk2 = pa.tile([16, 128], dt.int32, name="pk2", tag="pk2")
        nc.gpsimd.iota(pk2[:], pattern=[[1, 128]], base=0, channel_multiplier=1)
        dif = pa.tile([16, 128], dt.int32, name="dif", tag="dif")
        nc.vector.tensor_tensor(out=dif[:], in0=pk2[:], in1=c1[:], op=AL.subtract)
        # P16sel[kk, p] = 1 iff (p//32)*4 + (p%16)//4 == kk
        tgt = pa.tile([16, 128], dt.int32, name="tgt", tag="tgt")
        nc.vector.tensor_scalar(
            out=tgt[:], in0=c1[:], scalar1=5, scalar2=2,
            op0=AL.arith_shift_right, op1=AL.arith_shift_left,
        )
        tm = pa.tile([16, 128], dt.int32, name="tm", tag="tm")
        nc.vector.tensor_scalar(
            out=tm[:], in0=c1[:], scalar1=2, scalar2=3,
            op0=AL.arith_shift_right, op1=AL.bitwise_and,
        )
        nc.vector.tensor_tensor(out=tgt[:], in0=tgt[:], in1=tm[:], op=AL.add)
        nc.vector.tensor_tensor(out=tgt[:], in0=tgt[:], in1=dif[:], op=AL.subtract)
        p16sel = pool.tile([16, 128], dt.float32, name="p16sel")
        nc.vector.tensor_scalar(
            out=p16sel[:], in0=tgt[:], scalar1=0, scalar2=None, op0=AL.is_equal,
        )
        # P16b[kk, p] = 1 iff p//32 == kk//4
        nc.vector.tensor_scalar(
            out=tm[:], in0=c1[:], scalar1=5, scalar2=None,
            op0=AL.arith_shift_right,
        )
        t2 = pa.tile([16, 128], dt.int32, name="t2", tag="t2")
        nc.vector.tensor_scalar(
            out=t2[:], in0=dif[:], scalar1=2, scalar2=None,
            op0=AL.arith_shift_right,
        )
        nc.vector.tensor_tensor(out=tm[:], in0=tm[:], in1=t2[:], op=AL.subtract)
        p16b = pool.tile([16, 128], dt.float32, name="p16b")
        nc.vector.tensor_scalar(
            out=p16b[:], in0=tm[:], scalar1=0, scalar2=None, op0=AL.is_equal,
        )
        # Mdiag[kk, slo'] = 1 iff kk%4 == slo'
        c4 = pa.tile([16, 4], dt.int32, name="c4", tag="c4")
        nc.gpsimd.iota(c4[:], pattern=[[1, 4]], base=0, channel_multiplier=0)
        pk4 = pa.tile([16, 4], dt.int32, name="pk4", tag="pk4")
        nc.gpsimd.iota(pk4[:], pattern=[[1, 4]], base=0, channel_multiplier=1)
        nc.vector.tensor_tensor(out=pk4[:], in0=pk4[:], in1=c4[:], op=AL.subtract)
        nc.vector.tensor_scalar(
            out=pk4[:], in0=pk4[:], scalar1=3, scalar2=None, op0=AL.bitwise_and,
        )
        nc.vector.tensor_tensor(out=pk4[:], in0=pk4[:], in1=c4[:], op=AL.subtract)
        mdiag = pool.tile([16, 4], dt.float32, name="mdiag")
        nc.vector.tensor_scalar(
            out=mdiag[:], in0=pk4[:], scalar1=0, scalar2=None, op0=AL.is_equal,
        )

        # ---- boxes: partition (b, slo) holds its 8 points' (n, c) rows ----
        bx16 = pool.tile([16, 32], dt.float32)
        bx_in = boxes.rearrange("b (s4 slo) c -> b slo s4 c", slo=4)
        for b in range(BL):
            nc.sync.dma_start(out=bx16[4 * b:4 * (b + 1), :], in_=bx_in[b])

        # ---- feature streams: plain [128, E] contiguous, 2 rings ----
        T_tiles = []
        for li, (H, W) in enumerate(LEVELS):
            E = H * W * 8
            T = pstream.tile([128, E], dt.float32, name=f"T{li}")
            nc.sync.dma_start(out=T[:, 0:E // 2], in_=feats[li].ap()[:, 0:E // 2])
            nc.scalar.dma_start(out=T[:, E // 2:E], in_=feats[li].ap()[:, E // 2:E])
            T_tiles.append(T)

        # ---- Phase A (per level): point math on 16 partitions ----
        bxv = bx16[:].rearrange("p (s c) -> p s c", c=4)
        coord2 = bxv[:, :, 0:2]  # [16, 8, 2] (cx, cy)

        widxs, wbs = [], []
        for li, (H, W) in enumerate(LEVELS):
            HW = H * W
            wlog = W.bit_length() - 1
            E1 = float(W - 1)

            # pf = clip(coord*(E-1), 0, E-1); e0 = clamp(floor(pf), 0, E-2);
            # we = pf - e0.  floor via 16.16 fixed point.
            pf2 = pa.tile([16, 16], dt.float32, name="pf2", tag="pf2")
            nc.vector.tensor_scalar(
                out=pf2[:].rearrange("p (s c) -> p s c", c=2), in0=coord2,
                scalar1=E1, scalar2=0.0, op0=AL.mult, op1=AL.max,
            )
            nc.vector.tensor_scalar_min(out=pf2[:], in0=pf2[:], scalar1=E1)
            ifx2 = pa.tile([16, 16], dt.int32, name="ifx2", tag="ifx2")
            nc.vector.tensor_scalar(
                out=ifx2[:], in0=pf2[:], scalar1=65536.0, scalar2=None,
                op0=AL.mult,
            )
            nc.vector.tensor_scalar(
                out=ifx2[:], in0=ifx2[:], scalar1=16, scalar2=None,
                op0=AL.arith_shift_right,
            )
            e02 = pa.tile([16, 16], dt.float32, name="e02", tag="e02")
            nc.vector.tensor_scalar_min(out=e02[:], in0=ifx2[:], scalar1=float(W - 2))
            we2 = pa.tile([16, 16], dt.float32, name="we2", tag="we2")
            nc.vector.tensor_tensor(out=we2[:], in0=pf2[:], in1=e02[:], op=AL.subtract)
            w12 = pa.tile([16, 16], dt.float32, name="w12", tag="w12")
            nc.vector.tensor_scalar(
                out=w12[:], in0=we2[:], scalar1=-1.0, scalar2=1.0,
                op0=AL.mult, op1=AL.add,
            )
            e02v = e02[:].rearrange("p (s c) -> p s c", c=2)
            we2v = we2[:].rearrange("p (s c) -> p s c", c=2)
            w12v = w12[:].rearrange("p (s c) -> p s c", c=2)
            x0f, y0f = e02v[:, :, 0], e02v[:, :, 1]
            wx, wy = we2v[:, :, 0], we2v[:, :, 1]
            w1x, w1y = w12v[:, :, 0], w12v[:, :, 1]

            # w16[(b,slo), (s4, k)] = wyk * wxk
            w16 = pa.tile([16, 32], dt.float32, name="w16", tag="w16")
            w16v = w16[:].rearrange("p (s k) -> p s k", k=4)
            for k, (wyt, wxt) in enumerate(
                [(w1y, w1x), (w1y, wx), (wy, w1x), (wy, wx)]
            ):
                nc.vector.tensor_tensor(
                    out=w16v[:, :, k], in0=wyt, in1=wxt, op=AL.mult,
                )
            # rhs16[(b,slo), (s4, slo', k)] = w16[(b,slo), (s4, k)] * (slo'==slo)
            rhs16 = pa.tile([16, 128], dt.float32, name="rhs16", tag="rhs16")
            nc.vector.tensor_tensor(
                out=rhs16[:].rearrange("p (s l k) -> p s l k", s=8, l=4),
                in0=w16v.unsqueeze(2).to_broadcast([16, 8, 4, 4]),
                in1=mdiag[:].unsqueeze(1).unsqueeze(3).to_broadcast([16, 8, 4, 4]),
                op=AL.mult,
            )
            # wb[p, (s4, slo, k)] = w(p//32, s, k)
            wb_ps = ppsum.tile([128, 128], dt.float32, name=f"wbps{li}", tag="wbps")
            nc.tensor.matmul(wb_ps[:], p16b[:], rhs16[:], start=True, stop=True)
            wb = pool.tile([128, 128], dt.float32, name=f"wb{li}")
            nc.vector.tensor_copy(out=wb[:], in_=wb_ps[:])
            wbs.append(wb)

            # base16[(b,slo), s4] = y0*W + x0
            base16 = pa.tile([16, 8], dt.float32, name="base16", tag="base16")
            nc.vector.tensor_scalar(
                out=base16[:], in0=y0f, scalar1=float(W), scalar2=None,
                op0=AL.mult,
            )
            nc.vector.tensor_tensor(out=base16[:], in0=base16[:], in1=x0f, op=AL.add)
            # basefP[p, s4] = base16[(p//32)*4 + (p%16)//4, s4]
            bp_ps = ppsum.tile([128, 8], dt.float32, name=f"bpps{li}", tag="bpps")
            nc.tensor.matmul(bp_ps[:], p16sel[:], base16[:], start=True, stop=True)

            # dk1[p] = ((p>>1)&1)*W + (p&1)   (k = p%4 corner offset)
            dk1 = pa.tile([128, 1], dt.int32, name="dk1", tag="dk1")
            nc.vector.tensor_scalar(
                out=dk1[:], in0=pid[:], scalar1=1, scalar2=1,
                op0=AL.arith_shift_right, op1=AL.bitwise_and,
            )
            nc.vector.tensor_scalar(
                out=dk1[:], in0=dk1[:], scalar1=wlog, scalar2=None,
                op0=AL.logical_shift_left,
            )
            xk1 = pa.tile([128, 1], dt.int32, name="xk1", tag="xk1")
            nc.vector.tensor_scalar(
                out=xk1[:], in0=pid[:], scalar1=1, scalar2=None,
                op0=AL.bitwise_and,
            )
            nc.vector.tensor_tensor(out=dk1[:], in0=dk1[:], in1=xk1[:], op=AL.add)
            dk1f = pa.tile([128, 1], dt.float32, name="dk1f", tag="dk1f")
            nc.vector.tensor_copy(out=dk1f[:], in_=dk1[:])

            # widx[p, s4] = basefP[p, s4] + dk1[p]
            widxf = pa.tile([128, 8], dt.float32, name="widxf", tag="widxf")
            nc.vector.tensor_tensor(
                out=widxf[:], in0=bp_ps[:],
                in1=dk1f[:].to_broadcast([128, 8]), op=AL.add,
            )
            widx = pool.tile([128, 8], dt.int16, name=f"widx{li}")
            nc.vector.tensor_copy(out=widx[:], in_=widxf[:])
            widxs.append(widx)

        # ---- gathers (one per level, d=8) + lerp ----
        V = [pool.tile([128, NPTS * 2], dt.float32, name=f"V{li}") for li in range(3)]
        for li, (H, W) in enumerate(LEVELS):
            HW = H * W
            og = pwork.tile([128, 1024], dt.float32, name=f"og{li}", tag="og")
            nc.gpsimd.ap_gather(
                out_ap=og[:], in_ap=T_tiles[li][:], idxs_ap=widxs[li][:],
                channels=128, num_elems=HW, d=8, num_idxs=128,
            )
            # weights: col (s4, slo, k, jj): w(b, s, k) broadcast over jj
            og_v = og[:].rearrange("c (j jj) -> c j jj", jj=8)
            wb_bc = wbs[li][:].unsqueeze(2).to_broadcast([128, 128, 8])
            nc.vector.tensor_tensor(out=og_v, in0=og_v, in1=wb_bc, op=AL.mult)
            # corner sum over k (middle axis): V[p, (s, jj)] = sum_k og
            ogk = og[:].rearrange("c (s k jj) -> c s k jj", s=32, k=4)
            nc.vector.tensor_tensor(
                out=V[li][:].rearrange("c (s jj) -> c s jj", s=32),
                in0=ogk[:, :, 0], in1=ogk[:, :, 1], op=AL.add,
            )
            nc.vector.tensor_tensor(
                out=V[li][:].rearrange("c (s jj) -> c s jj", s=32),
                in0=V[li][:].rearrange("c (s jj) -> c s jj", s=32),
                in1=ogk[:, :, 2], op=AL.add,
            )
            nc.vector.tensor_tensor(
                out=V[li][:].rearrange("c (s jj) -> c s jj", s=32),
                in0=V[li][:].rearrange("c (s jj) -> c s jj", s=32),
                in1=ogk[:, :, 3], op=AL.add,
            )

        # ---- per-point channel sums: partitions contract via P4sel matmul.
        def colsum(name, vi, vj):
            prod = pwork.tile([128, NPTS * 2], dt.float32, name=f"prod{name}", tag="og")
            nc.vector.tensor_tensor(out=prod[:], in0=vi[:], in1=vj[:], op=AL.mult)
            ps = ppsum.tile([4, NPTS * 2], dt.float32, name=name, tag="cs")
            nc.tensor.matmul(ps[:], p4sel[:], prod[:], start=True, stop=True)
            sb = pool.tile([4, 32], dt.float32, name=f"sb{name}")
            nc.vector.tensor_reduce(
                out=sb[:], in_=ps[:].rearrange("p (s jj) -> p s jj", jj=8),
                axis=mybir.AxisListType.X, op=AL.add,
            )
            return sb

        ss = [colsum(f"ss{li}", V[li], V[li]) for li in range(3)]
        dots = {(i, j): colsum(f"d{i}{j}", V[i], V[j]) for i, j in PAIRS}

        # ---- cosine epilogue on [4, 32] ----
        rns = []
        for li in range(3):
            nrm = pool.tile([4, 32], dt.float32, name=f"nrm{li}")
            nc.scalar.sqrt(out=nrm[:], in_=ss[li][:])
            nc.vector.tensor_scalar_max(out=nrm[:], in0=nrm[:], scalar1=EPS)
            rn = pool.tile([4, 32], dt.float32, name=f"rn{li}")
            nc.vector.reciprocal(out=rn[:], in_=nrm[:])
            rns.append(rn)

        tot = pool.tile([4, 32], dt.float32)
        first = True
        for i, j in PAIRS:
            t = pool.tile([4, 32], dt.float32, name=f"t{i}{j}")
            nc.vector.tensor_tensor(
                out=t[:], in0=dots[(i, j)][:], in1=rns[i][:], op=AL.mult
            )
            nc.vector.tensor_tensor(out=t[:], in0=t[:], in1=rns[j][:], op=AL.mult)
            if first:
                nc.vector.tensor_copy(out=tot[:], in_=t[:])
                first = False
            else:
                nc.vector.tensor_tensor(out=tot[:], in0=tot[:], in1=t[:], op=AL.add)

        tot4 = pool.tile([4, 1], dt.float32)
        nc.vector.tensor_reduce(
            out=tot4[:], in_=tot[:], axis=mybir.AxisListType.X, op=AL.add
        )
        res_ps = ppsum.tile([1, 1], dt.float32, name="resps")
        nc.tensor.matmul(res_ps[:], tot4[:], ones4[:], start=True, stop=True)
        res = pool.tile([1, 1], dt.float32)
        nc.vector.tensor_copy(out=res[:], in_=res_ps[:])
        nc.sync.dma_start(out=out.ap(), in_=res[:])

    nc.compile()
    return nc


def _get_program():
    if "nc" not in _CACHE:
        _CACHE["nc"] = _build_program()
    return _CACHE["nc"]


def _prep_feats(feat0, feat1, feat2):
    """Host-side layout: per level, per core, [128, H*W*8] with partition
    p = (b = p//32, q = p%32) holding channels 8q..8q+7 CHANNEL-LAST
    ([H*W, 8] per partition) so the d=8 gather fetches one corner's 8
    channel values as a contiguous run."""
    outs = []
    for li, f in enumerate((feat0, feat1, feat2)):
        H, W = LEVELS[li]
        HW = H * W
        a = np.asarray(f, dtype=np.float32).reshape(B, 32, 8, HW)
        a = np.ascontiguousarray(a.transpose(0, 1, 3, 2))  # [B, 32, HW, 8]
        outs.append(a.reshape(B, 32, HW * 8))
    return outs


def _run_device(feat0, feat1, feat2, boxes, **run_kwargs):
    """Shard inputs batch-wise over the 8 cores, run the SPMD program, and
    return the BassKernelResults (one {"out": [1,1]} per core)."""
    from concourse.bass_utils import run_bass_kernel_spmd

    nc = _get_program()
    feats_t = _prep_feats(feat0, feat1, feat2)
    boxes = np.ascontiguousarray(np.asarray(boxes, dtype=np.float32))

    in_maps = []
    for k in range(N_CORES):
        sl = slice(k * BL, (k + 1) * BL)
        in_maps.append(
            {
                "feat0": feats_t[0][sl].reshape(128, -1),
                "feat1": feats_t[1][sl].reshape(128, -1),
                "feat2": feats_t[2][sl].reshape(128, -1),
                "boxes": boxes[sl],
            }
        )

    return run_bass_kernel_spmd(
        nc, in_maps, core_ids=list(range(N_CORES)), **run_kwargs
    )


def kernel(feat0, feat1, feat2, boxes):
    r = _run_device(feat0, feat1, feat2, boxes)
    total = np.float64(0.0)
    for m in r.results:
        total += np.float64(m["out"].reshape(-1)[0])

    count = B * N * len(PAIRS)
    avg = np.float32(total) / np.float32(count)
    loss = np.float32(1.0) - avg
    loss = np.nan_to_num(loss, nan=0.0, posinf=1.0, neginf=0.0)
    return np.array(np.clip(loss, 0.0, 2.0), dtype=np.float32)


# revision 29
# speedup vs baseline: 2.0470x; 1.0439x over previous
"""Trainium2 Bass kernel for nn_CSCLoss: multi-scale bilinear point-sampling
cosine-consistency loss.

loss = 1 - mean_{pairs,(b,n)} <normalize(sample(feat_i, p_bn)), normalize(sample(feat_j, p_bn))>

Sharding: data-parallel over batch — 32 images -> 8 cores x 4 images; the
host sums the 8 per-core partial sums and applies the 1 - total/count
epilogue (the all-reduce of the sharding hint, done on 8 scalars).

Key structural facts (HW-measured):
 - ap_gather costs ~30 Q7 cycles PER OUTPUT COLUMN regardless of
   batching; minimizing gather columns is the only lever.  Columns drop
   8x by gathering with d=8: the HOST pre-arranges each level as
   [128, H*W, 8] per core — partition p = (image b = p//32, chunk
   q = p%32) holds channels 8q..8q+7 CHANNEL-LAST, so one index fetches
   a corner's 8 channel values as one contiguous 32 B run.  128 columns
   per level (32 points x 4 corners), 384 total (~12 us on Pool).
 - The DMA streams the pre-arranged arrays as plain contiguous [128, E]
   tiles split across both HWDGE rings -> HBM line rate, big level
   first.  num_elems = H*W = 4096 (lvl0) meets ap_gather's
   num_elems*d*4/4 <= 2^15 limit exactly.
 - SWDGE DMA round trips cost 10-15 us under the feature stream, so
   staging uses NONE: boxes load as [16, 32] (partition (b, slo) holds
   the 8 points s = 4*s4+slo), point math runs on 16 partitions, and
   static iota-built selector matmuls (P16sel / P16b) replicate index
   bases and corner weights to the [128, *] gather layout.
 - Gather-out col j = s4*16 + slo*4 + k at partition group r = j%16
   (slo = r//4, k = r%4), cb = j//16 = s4: idx = y0*W + x0 + dk(k),
   image-local.  Weights w(b, s, k) broadcast over the d=8 channel dim.
 - Per-point channel sums: V_i*V_j products, P4sel matmul contracts
   partitions -> PSUM [4, (s, jj)], reduce jj -> [4, 32] per point.
   Cosine epilogue on [4, 32]; final cross-image sum via a [4,1]x[4,1]
   matmul; one [1,1] DMA out per core.
"""

import sys
from contextlib import ExitStack

import numpy as np

if "/opt/trn_rl_repo" not in sys.path:
    sys.path.insert(0, "/opt/trn_rl_repo")

B, N, C = 32, 32, 256
LEVELS = [(64, 64), (32, 32), (16, 16)]  # (H, W), all square
N_CORES = 8
BL = B // N_CORES          # images per core
NPTS = BL * N              # 128 points per core
PAIRS = [(0, 1), (0, 2), (1, 2)]
EPS = 1e-12

_CACHE = {}


def _build_program():
    from concourse import bacc, bass, mybir, tile, library_config

    dt = mybir.dt
    AL = mybir.AluOpType

    nc = bacc.Bacc("TRN2", target_bir_lowering=False, debug=False)

    feats = [
        nc.dram_tensor(
            f"feat{i}", [128, H * W * 8], dt.float32, kind="ExternalInput"
        )
        for i, (H, W) in enumerate(LEVELS)
    ]
    boxes = nc.dram_tensor("boxes", [BL, N, 4], dt.float32, kind="ExternalInput")
    c16 = nc.dram_tensor("c16", [16, 260], dt.float32, kind="ExternalInput")
    cP = nc.dram_tensor("cP", [128, 3], dt.float32, kind="ExternalInput")
    out = nc.dram_tensor("out", [1, 1], dt.float32, kind="ExternalOutput")

    with tile.TileContext(nc) as tc, ExitStack() as ctx:
        pool = ctx.enter_context(tc.tile_pool(name="sbuf", bufs=1))
        pa = ctx.enter_context(tc.tile_pool(name="pa", bufs=1))
        pstream = ctx.enter_context(tc.tile_pool(name="stream", bufs=1))
        pwork = ctx.enter_context(tc.tile_pool(name="work", bufs=2))
        ppsum = ctx.enter_context(tc.tile_pool(name="psum", bufs=1, space="PSUM"))

        nc.gpsimd.load_library(library_config.ap_gather)

        # ---- static setup ----
        # P4sel[p, m] = 1.0 iff p//32 == m (colsum lhsT, bf16 for 1-pass
        # matmuls; 32-aligned memsets).  All other static selector tables
        # (p16sel, p16b, mdiag, per-level dk offsets) are host-precomputed
        # constants DMA'd in up front — NO gpsimd iotas, so the Pool queue
        # needs only the ap_gather library (no mid-kernel library switch,
        # whose critical section stalls the HWDGE rings for ~15 us).
        p4sel = pool.tile([128, 4], dt.bfloat16)
        nc.vector.memset(p4sel[:], 0.0)
        for m in range(4):
            nc.vector.memset(p4sel[32 * m:32 * (m + 1), m:m + 1], 1.0)
        ones4 = pool.tile([4, 1], dt.float32)
        nc.vector.memset(ones4[:], 1.0)
        c16t = pool.tile([16, 260], dt.float32, name="c16t")
        nc.sync.dma_start(out=c16t[:], in_=c16.ap())
        cPt = pool.tile([128, 3], dt.float32, name="cPt")
        nc.sync.dma_start(out=cPt[:], in_=cP.ap())
        p16sel = c16t[:, 0:128]
        p16b = c16t[:, 128:256]
        mdiag = c16t[:, 256:260]

        # ---- boxes: partition (b, slo) holds its 8 points' (n, c) rows ----
        bx16 = pool.tile([16, 32], dt.float32)
        bx_in = boxes.rearrange("b (s4 slo) c -> b slo s4 c", slo=4)
        for b in range(BL):
            nc.sync.dma_start(out=bx16[4 * b:4 * (b + 1), :], in_=bx_in[b])

        # ---- feature streams: plain [128, E] contiguous, 2 rings ----
        T_tiles = []
        for li, (H, W) in enumerate(LEVELS):
            E = H * W * 8
            T = pstream.tile([128, E], dt.float32, name=f"T{li}")
            nc.sync.dma_start(out=T[:, 0:E // 2], in_=feats[li].ap()[:, 0:E // 2])
            nc.scalar.dma_start(out=T[:, E // 2:E], in_=feats[li].ap()[:, E // 2:E])
            T_tiles.append(T)

        # ---- Phase A (per level): point math on 16 partitions ----
        bxv = bx16[:].rearrange("p (s c) -> p s c", c=4)
        coord2 = bxv[:, :, 0:2]  # [16, 8, 2] (cx, cy)

        widxs, wbs = [], []
        for li, (H, W) in enumerate(LEVELS):
            HW = H * W
            E1 = float(W - 1)

            # pf = clip(coord*(E-1), 0, E-1); e0 = clamp(floor(pf), 0, E-2);
            # we = pf - e0.  floor via 16.16 fixed point.
            pf2 = pa.tile([16, 16], dt.float32, name="pf2", tag="pf2")
            nc.vector.tensor_scalar(
                out=pf2[:].rearrange("p (s c) -> p s c", c=2), in0=coord2,
                scalar1=E1, scalar2=0.0, op0=AL.mult, op1=AL.max,
            )
            nc.vector.tensor_scalar_min(out=pf2[:], in0=pf2[:], scalar1=E1)
            ifx2 = pa.tile([16, 16], dt.int32, name="ifx2", tag="ifx2")
            nc.vector.tensor_scalar(
                out=ifx2[:], in0=pf2[:], scalar1=65536.0, scalar2=None,
                op0=AL.mult,
            )
            nc.vector.tensor_scalar(
                out=ifx2[:], in0=ifx2[:], scalar1=16, scalar2=None,
                op0=AL.arith_shift_right,
            )
            e02 = pa.tile([16, 16], dt.float32, name="e02", tag="e02")
            nc.vector.tensor_scalar_min(out=e02[:], in0=ifx2[:], scalar1=float(W - 2))
            we2 = pa.tile([16, 16], dt.float32, name="we2", tag="we2")
            nc.vector.tensor_tensor(out=we2[:], in0=pf2[:], in1=e02[:], op=AL.subtract)
            w12 = pa.tile([16, 16], dt.float32, name="w12", tag="w12")
            nc.vector.tensor_scalar(
                out=w12[:], in0=we2[:], scalar1=-1.0, scalar2=1.0,
                op0=AL.mult, op1=AL.add,
            )
            e02v = e02[:].rearrange("p (s c) -> p s c", c=2)
            we2v = we2[:].rearrange("p (s c) -> p s c", c=2)
            w12v = w12[:].rearrange("p (s c) -> p s c", c=2)
            x0f, y0f = e02v[:, :, 0], e02v[:, :, 1]
            wx, wy = we2v[:, :, 0], we2v[:, :, 1]
            w1x, w1y = w12v[:, :, 0], w12v[:, :, 1]

            # w16[(b,slo), (s4, k)] = wyk * wxk
            w16 = pa.tile([16, 32], dt.float32, name="w16", tag="w16")
            w16v = w16[:].rearrange("p (s k) -> p s k", k=4)
            for k, (wyt, wxt) in enumerate(
                [(w1y, w1x), (w1y, wx), (wy, w1x), (wy, wx)]
            ):
                nc.vector.tensor_tensor(
                    out=w16v[:, :, k], in0=wyt, in1=wxt, op=AL.mult,
                )
            # rhs16[(b,slo), (s4, slo', k)] = w16[(b,slo), (s4, k)] * (slo'==slo)
            rhs16 = pa.tile([16, 128], dt.float32, name="rhs16", tag="rhs16")
            nc.vector.tensor_tensor(
                out=rhs16[:].rearrange("p (s l k) -> p s l k", s=8, l=4),
                in0=w16v.unsqueeze(2).to_broadcast([16, 8, 4, 4]),
                in1=mdiag.unsqueeze(1).unsqueeze(3).to_broadcast([16, 8, 4, 4]),
                op=AL.mult,
            )
            # wb[p, (s4, slo, k)] = w(p//32, s, k)
            wb_ps = ppsum.tile([128, 128], dt.float32, name=f"wbps{li}", tag="wbps")
            nc.tensor.matmul(wb_ps[:], p16b, rhs16[:], start=True, stop=True)
            wb = pool.tile([128, 128], dt.float32, name=f"wb{li}")
            nc.vector.tensor_copy(out=wb[:], in_=wb_ps[:])
            wbs.append(wb)

            # base16[(b,slo), s4] = y0*W + x0
            base16 = pa.tile([16, 8], dt.float32, name="base16", tag="base16")
            nc.vector.tensor_scalar(
                out=base16[:], in0=y0f, scalar1=float(W), scalar2=None,
                op0=AL.mult,
            )
            nc.vector.tensor_tensor(out=base16[:], in0=base16[:], in1=x0f, op=AL.add)
            # basefP[p, s4] = base16[(p//32)*4 + (p%16)//4, s4]
            bp_ps = ppsum.tile([128, 8], dt.float32, name=f"bpps{li}", tag="bpps")
            nc.tensor.matmul(bp_ps[:], p16sel, base16[:], start=True, stop=True)

            # widx[p, s4] = basefP[p, s4] + dk1[p]
            # (dk1[p] = ((p>>1)&1)*W + (p&1), host-precomputed per level)
            widxf = pa.tile([128, 8], dt.float32, name="widxf", tag="widxf")
            nc.vector.tensor_tensor(
                out=widxf[:], in0=bp_ps[:],
                in1=cPt[:, li:li + 1].to_broadcast([128, 8]), op=AL.add,
            )
            widx = pool.tile([128, 8], dt.int16, name=f"widx{li}")
            nc.vector.tensor_copy(out=widx[:], in_=widxf[:])
            widxs.append(widx)

        # ---- gathers (one per level, d=8) + lerp ----
        V = [pool.tile([128, NPTS * 2], dt.float32, name=f"V{li}") for li in range(3)]
        for li, (H, W) in enumerate(LEVELS):
            HW = H * W
            og = pwork.tile([128, 1024], dt.float32, name=f"og{li}", tag="og")
            nc.gpsimd.ap_gather(
                out_ap=og[:], in_ap=T_tiles[li][:], idxs_ap=widxs[li][:],
                channels=128, num_elems=HW, d=8, num_idxs=128,
            )
            # weights: col (s4, slo, k, jj): w(b, s, k) broadcast over jj
            og_v = og[:].rearrange("c (j jj) -> c j jj", jj=8)
            wb_bc = wbs[li][:].unsqueeze(2).to_broadcast([128, 128, 8])
            nc.vector.tensor_tensor(out=og_v, in0=og_v, in1=wb_bc, op=AL.mult)
            # corner sum over k (middle axis): V[p, (s, jj)] = sum_k og
            ogk = og[:].rearrange("c (s k jj) -> c s k jj", s=32, k=4)
            nc.vector.tensor_tensor(
                out=V[li][:].rearrange("c (s jj) -> c s jj", s=32),
                in0=ogk[:, :, 0], in1=ogk[:, :, 1], op=AL.add,
            )
            nc.vector.tensor_tensor(
                out=V[li][:].rearrange("c (s jj) -> c s jj", s=32),
                in0=V[li][:].rearrange("c (s jj) -> c s jj", s=32),
                in1=ogk[:, :, 2], op=AL.add,
            )
            nc.vector.tensor_tensor(
                out=V[li][:].rearrange("c (s jj) -> c s jj", s=32),
                in0=V[li][:].rearrange("c (s jj) -> c s jj", s=32),
                in1=ogk[:, :, 3], op=AL.add,
            )

        # ---- per-point channel sums: partitions contract via P4sel matmul.
        _csn = [0]

        def colsum(name, vi, vj):
            prod = pwork.tile([128, NPTS * 2], dt.bfloat16, name=f"prod{name}", tag="og")
            nc.vector.tensor_tensor(out=prod[:], in0=vi[:], in1=vj[:], op=AL.mult)
            _csn[0] += 1
            ps = ppsum.tile([4, NPTS * 2], dt.float32, name=name, tag=f"cs{_csn[0] % 2}")
            nc.tensor.matmul(ps[:], p4sel[:], prod[:], start=True, stop=True)
            sb = pool.tile([4, 32], dt.float32, name=f"sb{name}")
            nc.vector.tensor_reduce(
                out=sb[:], in_=ps[:].rearrange("p (s jj) -> p s jj", jj=8),
                axis=mybir.AxisListType.X, op=AL.add,
            )
            return sb

        ss = [colsum(f"ss{li}", V[li], V[li]) for li in range(3)]
        dots = {(i, j): colsum(f"d{i}{j}", V[i], V[j]) for i, j in PAIRS}

        # ---- cosine epilogue on [4, 32] ----
        rns = []
        for li in range(3):
            nrm = pool.tile([4, 32], dt.float32, name=f"nrm{li}")
            nc.scalar.sqrt(out=nrm[:], in_=ss[li][:])
            nc.vector.tensor_scalar_max(out=nrm[:], in0=nrm[:], scalar1=EPS)
            rn = pool.tile([4, 32], dt.float32, name=f"rn{li}")
            nc.vector.reciprocal(out=rn[:], in_=nrm[:])
            rns.append(rn)

        tot = pool.tile([4, 32], dt.float32)
        first = True
        for i, j in PAIRS:
            t = pool.tile([4, 32], dt.float32, name=f"t{i}{j}")
            nc.vector.tensor_tensor(
                out=t[:], in0=dots[(i, j)][:], in1=rns[i][:], op=AL.mult
            )
            nc.vector.tensor_tensor(out=t[:], in0=t[:], in1=rns[j][:], op=AL.mult)
            if first:
                nc.vector.tensor_copy(out=tot[:], in_=t[:])
                first = False
            else:
                nc.vector.tensor_tensor(out=tot[:], in0=tot[:], in1=t[:], op=AL.add)

        tot4 = pool.tile([4, 1], dt.float32)
        nc.vector.tensor_reduce(
            out=tot4[:], in_=tot[:], axis=mybir.AxisListType.X, op=AL.add
        )
        res_ps = ppsum.tile([1, 1], dt.float32, name="resps")
        nc.tensor.matmul(res_ps[:], tot4[:], ones4[:], start=True, stop=True)
        res = pool.tile([1, 1], dt.float32)
        nc.vector.tensor_copy(out=res[:], in_=res_ps[:])
        nc.sync.dma_start(out=out.ap(), in_=res[:])

    nc.compile()
    return nc


def _get_program():
    if "nc" not in _CACHE:
        _CACHE["nc"] = _build_program()
    return _CACHE["nc"]


def _prep_feats(feat0, feat1, feat2):
    """Host-side layout: per level, per core, [128, H*W*8] with partition
    p = (b = p//32, q = p%32) holding channels 8q..8q+7 CHANNEL-LAST
    ([H*W, 8] per partition) so the d=8 gather fetches one corner's 8
    channel values as a contiguous run."""
    outs = []
    for li, f in enumerate((feat0, feat1, feat2)):
        H, W = LEVELS[li]
        HW = H * W
        a = np.asarray(f, dtype=np.float32).reshape(B, 32, 8, HW)
        a = np.ascontiguousarray(a.transpose(0, 1, 3, 2))  # [B, 32, HW, 8]
        outs.append(a.reshape(B, 32, HW * 8))
    return outs


def _run_device(feat0, feat1, feat2, boxes, **run_kwargs):
    """Shard inputs batch-wise over the 8 cores, run the SPMD program, and
    return the BassKernelResults (one {"out": [1,1]} per core)."""
    from concourse.bass_utils import run_bass_kernel_spmd

    nc = _get_program()
    feats_t = _prep_feats(feat0, feat1, feat2)
    boxes = np.ascontiguousarray(np.asarray(boxes, dtype=np.float32))

    # static selector tables (identical on every core)
    kk = np.arange(16)[:, None]
    p = np.arange(128)[None, :]
    c16 = np.zeros((16, 260), dtype=np.float32)
    c16[:, 0:128] = ((p // 32) * 4 + (p % 16) // 4 == kk)      # p16sel
    c16[:, 128:256] = (p // 32 == kk // 4)                     # p16b
    c16[:, 256:260] = (kk % 4 == np.arange(4)[None, :])        # mdiag
    pp = np.arange(128)
    cP = np.stack(
        [((pp >> 1) & 1) * W + (pp & 1) for (_, W) in LEVELS], axis=1
    ).astype(np.float32)                                       # dk1 per level

    in_maps = []
    for k in range(N_CORES):
        sl = slice(k * BL, (k + 1) * BL)
        in_maps.append(
            {
                "feat0": feats_t[0][sl].reshape(128, -1),
                "feat1": feats_t[1][sl].reshape(128, -1),
                "feat2": feats_t[2][sl].reshape(128, -1),
                "boxes": boxes[sl],
                "c16": c16,
                "cP": cP,
            }
        )

    return run_bass_kernel_spmd(
        nc, in_maps, core_ids=list(range(N_CORES)), **run_kwargs
    )


def kernel(feat0, feat1, feat2, boxes):
    r = _run_device(feat0, feat1, feat2, boxes)
    total = np.float64(0.0)
    for m in r.results:
        total += np.float64(m["out"].reshape(-1)[0])

    count = B * N * len(PAIRS)
    avg = np.float32(total) / np.float32(count)
    loss = np.float32(1.0) - avg
    loss = np.nan_to_num(loss, nan=0.0, posinf=1.0, neginf=0.0)
    return np.array(np.clip(loss, 0.0, 2.0), dtype=np.float32)


# revision 30
# speedup vs baseline: 2.9790x; 1.4553x over previous
"""Trainium2 Bass kernel for nn_CSCLoss: multi-scale bilinear point-sampling
cosine-consistency loss.

loss = 1 - mean_{pairs,(b,n)} <normalize(sample(feat_i, p_bn)), normalize(sample(feat_j, p_bn))>

Sharding: data-parallel over batch — 32 images -> 8 cores x 4 images; the
host sums the 8 per-core partial sums and applies the 1 - total/count
epilogue (the all-reduce of the sharding hint, done on 8 scalars).

Key structural facts (HW-measured):
 - ap_gather costs ~30 Q7 cycles PER OUTPUT COLUMN regardless of
   batching; minimizing gather columns is the only lever.  Columns drop
   8x by gathering with d=8: the HOST pre-arranges each level as
   [128, H*W, 8] per core — partition p = (image b = p//32, chunk
   q = p%32) holds channels 8q..8q+7 CHANNEL-LAST, so one index fetches
   a corner's 8 channel values as one contiguous 32 B run.  128 columns
   per level (32 points x 4 corners), 384 total (~12 us on Pool).
 - The DMA streams the pre-arranged arrays as plain contiguous [128, E]
   tiles split across both HWDGE rings -> HBM line rate, big level
   first.  num_elems = H*W = 4096 (lvl0) meets ap_gather's
   num_elems*d*4/4 <= 2^15 limit exactly.
 - SWDGE DMA round trips cost 10-15 us under the feature stream, so
   staging uses NONE: boxes load as [16, 32] (partition (b, slo) holds
   the 8 points s = 4*s4+slo), point math runs on 16 partitions, and
   static iota-built selector matmuls (P16sel / P16b) replicate index
   bases and corner weights to the [128, *] gather layout.
 - Gather-out col j = s4*16 + slo*4 + k at partition group r = j%16
   (slo = r//4, k = r%4), cb = j//16 = s4: idx = y0*W + x0 + dk(k),
   image-local.  Weights w(b, s, k) broadcast over the d=8 channel dim.
 - Per-point channel sums: V_i*V_j products, P4sel matmul contracts
   partitions -> PSUM [4, (s, jj)], reduce jj -> [4, 32] per point.
   Cosine epilogue on [4, 32]; final cross-image sum via a [4,1]x[4,1]
   matmul; one [1,1] DMA out per core.
"""

import sys
from contextlib import ExitStack

import ml_dtypes
import numpy as np

if "/opt/trn_rl_repo" not in sys.path:
    sys.path.insert(0, "/opt/trn_rl_repo")

B, N, C = 32, 32, 256
LEVELS = [(64, 64), (32, 32), (16, 16)]  # (H, W), all square
N_CORES = 8
BL = B // N_CORES          # images per core
NPTS = BL * N              # 128 points per core
PAIRS = [(0, 1), (0, 2), (1, 2)]
EPS = 1e-12

_CACHE = {}


def _build_program():
    from concourse import bacc, bass, mybir, tile, library_config

    dt = mybir.dt
    AL = mybir.AluOpType

    nc = bacc.Bacc("TRN2", target_bir_lowering=False, debug=False)

    feats = [
        nc.dram_tensor(
            f"feat{i}", [128, H * W * 8], dt.bfloat16, kind="ExternalInput"
        )
        for i, (H, W) in enumerate(LEVELS)
    ]
    boxes = nc.dram_tensor("boxes", [BL, N, 4], dt.float32, kind="ExternalInput")
    c16 = nc.dram_tensor("c16", [16, 260], dt.float32, kind="ExternalInput")
    cP = nc.dram_tensor("cP", [128, 3], dt.float32, kind="ExternalInput")
    out = nc.dram_tensor("out", [1, 1], dt.float32, kind="ExternalOutput")

    with tile.TileContext(nc) as tc, ExitStack() as ctx:
        pool = ctx.enter_context(tc.tile_pool(name="sbuf", bufs=1))
        pa = ctx.enter_context(tc.tile_pool(name="pa", bufs=1))
        pstream = ctx.enter_context(tc.tile_pool(name="stream", bufs=1))
        pwork = ctx.enter_context(tc.tile_pool(name="work", bufs=2))
        ppsum = ctx.enter_context(tc.tile_pool(name="psum", bufs=1, space="PSUM"))

        nc.gpsimd.load_library(library_config.ap_gather)

        # ---- static setup ----
        # P4sel[p, m] = 1.0 iff p//32 == m (colsum lhsT, bf16 for 1-pass
        # matmuls; 32-aligned memsets).  All other static selector tables
        # (p16sel, p16b, mdiag, per-level dk offsets) are host-precomputed
        # constants DMA'd in up front — NO gpsimd iotas, so the Pool queue
        # needs only the ap_gather library (no mid-kernel library switch,
        # whose critical section stalls the HWDGE rings for ~15 us).
        p4sel = pool.tile([128, 4], dt.bfloat16)
        nc.vector.memset(p4sel[:], 0.0)
        for m in range(4):
            nc.vector.memset(p4sel[32 * m:32 * (m + 1), m:m + 1], 1.0)
        ones4 = pool.tile([4, 1], dt.float32)
        nc.vector.memset(ones4[:], 1.0)
        c16t = pool.tile([16, 260], dt.float32, name="c16t")
        nc.sync.dma_start(out=c16t[:], in_=c16.ap())
        cPt = pool.tile([128, 3], dt.float32, name="cPt")
        nc.sync.dma_start(out=cPt[:], in_=cP.ap())
        p16sel = c16t[:, 0:128]
        p16b = c16t[:, 128:256]
        mdiag = c16t[:, 256:260]

        # ---- boxes: partition (b, slo) holds its 8 points' (n, c) rows ----
        bx16 = pool.tile([16, 32], dt.float32)
        bx_in = boxes.rearrange("b (s4 slo) c -> b slo s4 c", slo=4)
        for b in range(BL):
            nc.sync.dma_start(out=bx16[4 * b:4 * (b + 1), :], in_=bx_in[b])

        # ---- feature streams (bf16): T0/T1 halves on the two HWDGE
        # rings (level-ordered so T0's completion sems fire mid-stream);
        # T2 rides the third (SWDGE) queue so its completion is
        # independent of the rings' tail.
        T_tiles = []
        for li, (H, W) in enumerate(LEVELS):
            E = H * W * 8
            T = pstream.tile([128, E], dt.bfloat16, name=f"T{li}")
            if li < 2:
                nc.sync.dma_start(out=T[:, 0:E // 2], in_=feats[li].ap()[:, 0:E // 2])
                nc.scalar.dma_start(out=T[:, E // 2:E], in_=feats[li].ap()[:, E // 2:E])
            else:
                nc.gpsimd.dma_start(out=T[:], in_=feats[li].ap())
            T_tiles.append(T)

        # ---- Phase A (per level): point math on 16 partitions ----
        bxv = bx16[:].rearrange("p (s c) -> p s c", c=4)
        coord2 = bxv[:, :, 0:2]  # [16, 8, 2] (cx, cy)

        widxs, wbs = [], []
        for li, (H, W) in enumerate(LEVELS):
            HW = H * W
            E1 = float(W - 1)

            # pf = clip(coord*(E-1), 0, E-1); e0 = clamp(floor(pf), 0, E-2);
            # we = pf - e0.  floor via 16.16 fixed point.
            pf2 = pa.tile([16, 16], dt.float32, name="pf2", tag="pf2")
            nc.vector.tensor_scalar(
                out=pf2[:].rearrange("p (s c) -> p s c", c=2), in0=coord2,
                scalar1=E1, scalar2=0.0, op0=AL.mult, op1=AL.max,
            )
            nc.vector.tensor_scalar_min(out=pf2[:], in0=pf2[:], scalar1=E1)
            ifx2 = pa.tile([16, 16], dt.int32, name="ifx2", tag="ifx2")
            nc.vector.tensor_scalar(
                out=ifx2[:], in0=pf2[:], scalar1=65536.0, scalar2=None,
                op0=AL.mult,
            )
            nc.vector.tensor_scalar(
                out=ifx2[:], in0=ifx2[:], scalar1=16, scalar2=None,
                op0=AL.arith_shift_right,
            )
            e02 = pa.tile([16, 16], dt.float32, name="e02", tag="e02")
            nc.vector.tensor_scalar_min(out=e02[:], in0=ifx2[:], scalar1=float(W - 2))
            we2 = pa.tile([16, 16], dt.float32, name="we2", tag="we2")
            nc.vector.tensor_tensor(out=we2[:], in0=pf2[:], in1=e02[:], op=AL.subtract)
            w12 = pa.tile([16, 16], dt.float32, name="w12", tag="w12")
            nc.vector.tensor_scalar(
                out=w12[:], in0=we2[:], scalar1=-1.0, scalar2=1.0,
                op0=AL.mult, op1=AL.add,
            )
            e02v = e02[:].rearrange("p (s c) -> p s c", c=2)
            we2v = we2[:].rearrange("p (s c) -> p s c", c=2)
            w12v = w12[:].rearrange("p (s c) -> p s c", c=2)
            x0f, y0f = e02v[:, :, 0], e02v[:, :, 1]
            wx, wy = we2v[:, :, 0], we2v[:, :, 1]
            w1x, w1y = w12v[:, :, 0], w12v[:, :, 1]

            # w16[(b,slo), (s4, k)] = wyk * wxk
            w16 = pa.tile([16, 32], dt.float32, name="w16", tag="w16")
            w16v = w16[:].rearrange("p (s k) -> p s k", k=4)
            for k, (wyt, wxt) in enumerate(
                [(w1y, w1x), (w1y, wx), (wy, w1x), (wy, wx)]
            ):
                nc.vector.tensor_tensor(
                    out=w16v[:, :, k], in0=wyt, in1=wxt, op=AL.mult,
                )
            # rhs16[(b,slo), (s4, slo', k)] = w16[(b,slo), (s4, k)] * (slo'==slo)
            rhs16 = pa.tile([16, 128], dt.float32, name="rhs16", tag="rhs16")
            nc.vector.tensor_tensor(
                out=rhs16[:].rearrange("p (s l k) -> p s l k", s=8, l=4),
                in0=w16v.unsqueeze(2).to_broadcast([16, 8, 4, 4]),
                in1=mdiag.unsqueeze(1).unsqueeze(3).to_broadcast([16, 8, 4, 4]),
                op=AL.mult,
            )
            # wb[p, (s4, slo, k)] = w(p//32, s, k)
            wb_ps = ppsum.tile([128, 128], dt.float32, name=f"wbps{li}", tag="wbps")
            nc.tensor.matmul(wb_ps[:], p16b, rhs16[:], start=True, stop=True)
            wb = pool.tile([128, 128], dt.bfloat16, name=f"wb{li}")
            nc.vector.tensor_copy(out=wb[:], in_=wb_ps[:])
            wbs.append(wb)

            # base16[(b,slo), s4] = y0*W + x0
            base16 = pa.tile([16, 8], dt.float32, name="base16", tag="base16")
            nc.vector.tensor_scalar(
                out=base16[:], in0=y0f, scalar1=float(W), scalar2=None,
                op0=AL.mult,
            )
            nc.vector.tensor_tensor(out=base16[:], in0=base16[:], in1=x0f, op=AL.add)
            # basefP[p, s4] = base16[(p//32)*4 + (p%16)//4, s4]
            bp_ps = ppsum.tile([128, 8], dt.float32, name=f"bpps{li}", tag="bpps")
            nc.tensor.matmul(bp_ps[:], p16sel, base16[:], start=True, stop=True)

            # widx[p, s4] = basefP[p, s4] + dk1[p]
            # (dk1[p] = ((p>>1)&1)*W + (p&1), host-precomputed per level)
            widxf = pa.tile([128, 8], dt.float32, name="widxf", tag="widxf")
            nc.vector.tensor_tensor(
                out=widxf[:], in0=bp_ps[:],
                in1=cPt[:, li:li + 1].to_broadcast([128, 8]), op=AL.add,
            )
            widx = pool.tile([128, 8], dt.int16, name=f"widx{li}")
            nc.vector.tensor_copy(out=widx[:], in_=widxf[:])
            widxs.append(widx)

        # ---- gathers (one per level, d=8) + lerp ----
        V = [pool.tile([128, NPTS * 2], dt.bfloat16, name=f"V{li}") for li in range(3)]
        for li, (H, W) in enumerate(LEVELS):
            HW = H * W
            og = pwork.tile([128, 1024], dt.bfloat16, name=f"og{li}", tag="og")
            nc.gpsimd.ap_gather(
                out_ap=og[:], in_ap=T_tiles[li][:], idxs_ap=widxs[li][:],
                channels=128, num_elems=HW, d=8, num_idxs=128,
            )
            # weights: col (s4, slo, k, jj): w(b, s, k) broadcast over jj
            og_v = og[:].rearrange("c (j jj) -> c j jj", jj=8)
            wb_bc = wbs[li][:].unsqueeze(2).to_broadcast([128, 128, 8])
            nc.vector.tensor_tensor(out=og_v, in0=og_v, in1=wb_bc, op=AL.mult)
            # corner sum over k (middle axis): V[p, (s, jj)] = sum_k og
            ogk = og[:].rearrange("c (s k jj) -> c s k jj", s=32, k=4)
            nc.vector.tensor_tensor(
                out=V[li][:].rearrange("c (s jj) -> c s jj", s=32),
                in0=ogk[:, :, 0], in1=ogk[:, :, 1], op=AL.add,
            )
            nc.vector.tensor_tensor(
                out=V[li][:].rearrange("c (s jj) -> c s jj", s=32),
                in0=V[li][:].rearrange("c (s jj) -> c s jj", s=32),
                in1=ogk[:, :, 2], op=AL.add,
            )
            nc.vector.tensor_tensor(
                out=V[li][:].rearrange("c (s jj) -> c s jj", s=32),
                in0=V[li][:].rearrange("c (s jj) -> c s jj", s=32),
                in1=ogk[:, :, 3], op=AL.add,
            )

        # ---- per-point channel sums: partitions contract via P4sel matmul.
        _csn = [0]

        def colsum(name, vi, vj):
            prod = pwork.tile([128, NPTS * 2], dt.bfloat16, name=f"prod{name}", tag="og")
            nc.vector.tensor_tensor(out=prod[:], in0=vi[:], in1=vj[:], op=AL.mult)
            _csn[0] += 1
            ps = ppsum.tile([4, NPTS * 2], dt.float32, name=name, tag=f"cs{_csn[0] % 2}")
            nc.tensor.matmul(ps[:], p4sel[:], prod[:], start=True, stop=True)
            sb = pool.tile([4, 32], dt.float32, name=f"sb{name}")
            nc.vector.tensor_reduce(
                out=sb[:], in_=ps[:].rearrange("p (s jj) -> p s jj", jj=8),
                axis=mybir.AxisListType.X, op=AL.add,
            )
            return sb

        ss = [colsum(f"ss{li}", V[li], V[li]) for li in range(3)]
        dots = {(i, j): colsum(f"d{i}{j}", V[i], V[j]) for i, j in PAIRS}

        # ---- cosine epilogue on [4, 32] ----
        rns = []
        for li in range(3):
            nrm = pool.tile([4, 32], dt.float32, name=f"nrm{li}")
            nc.scalar.sqrt(out=nrm[:], in_=ss[li][:])
            nc.vector.tensor_scalar_max(out=nrm[:], in0=nrm[:], scalar1=EPS)
            rn = pool.tile([4, 32], dt.float32, name=f"rn{li}")
            nc.vector.reciprocal(out=rn[:], in_=nrm[:])
            rns.append(rn)

        tot = pool.tile([4, 32], dt.float32)
        first = True
        for i, j in PAIRS:
            t = pool.tile([4, 32], dt.float32, name=f"t{i}{j}")
            nc.vector.tensor_tensor(
                out=t[:], in0=dots[(i, j)][:], in1=rns[i][:], op=AL.mult
            )
            nc.vector.tensor_tensor(out=t[:], in0=t[:], in1=rns[j][:], op=AL.mult)
            if first:
                nc.vector.tensor_copy(out=tot[:], in_=t[:])
                first = False
            else:
                nc.vector.tensor_tensor(out=tot[:], in0=tot[:], in1=t[:], op=AL.add)

        tot4 = pool.tile([4, 1], dt.float32)
        nc.vector.tensor_reduce(
            out=tot4[:], in_=tot[:], axis=mybir.AxisListType.X, op=AL.add
        )
        res_ps = ppsum.tile([1, 1], dt.float32, name="resps")
        nc.tensor.matmul(res_ps[:], tot4[:], ones4[:], start=True, stop=True)
        res = pool.tile([1, 1], dt.float32)
        nc.vector.tensor_copy(out=res[:], in_=res_ps[:])
        nc.sync.dma_start(out=out.ap(), in_=res[:])

    nc.compile()
    return nc


def _get_program():
    if "nc" not in _CACHE:
        _CACHE["nc"] = _build_program()
    return _CACHE["nc"]


def _prep_feats(feat0, feat1, feat2):
    """Host-side layout: per level, per core, [128, H*W*8] with partition
    p = (b = p//32, q = p%32) holding channels 8q..8q+7 CHANNEL-LAST
    ([H*W, 8] per partition) so the d=8 gather fetches one corner's 8
    channel values as a contiguous run."""
    outs = []
    for li, f in enumerate((feat0, feat1, feat2)):
        H, W = LEVELS[li]
        HW = H * W
        a = np.asarray(f, dtype=np.float32).reshape(B, 32, 8, HW)
        a = np.ascontiguousarray(a.transpose(0, 1, 3, 2))  # [B, 32, HW, 8]
        outs.append(a.reshape(B, 32, HW * 8).astype(ml_dtypes.bfloat16))
    return outs


def _run_device(feat0, feat1, feat2, boxes, **run_kwargs):
    """Shard inputs batch-wise over the 8 cores, run the SPMD program, and
    return the BassKernelResults (one {"out": [1,1]} per core)."""
    from concourse.bass_utils import run_bass_kernel_spmd

    nc = _get_program()
    feats_t = _prep_feats(feat0, feat1, feat2)
    boxes = np.ascontiguousarray(np.asarray(boxes, dtype=np.float32))

    # static selector tables (identical on every core)
    kk = np.arange(16)[:, None]
    p = np.arange(128)[None, :]
    c16 = np.zeros((16, 260), dtype=np.float32)
    c16[:, 0:128] = ((p // 32) * 4 + (p % 16) // 4 == kk)      # p16sel
    c16[:, 128:256] = (p // 32 == kk // 4)                     # p16b
    c16[:, 256:260] = (kk % 4 == np.arange(4)[None, :])        # mdiag
    pp = np.arange(128)
    cP = np.stack(
        [((pp >> 1) & 1) * W + (pp & 1) for (_, W) in LEVELS], axis=1
    ).astype(np.float32)                                       # dk1 per level

    in_maps = []
    for k in range(N_CORES):
        sl = slice(k * BL, (k + 1) * BL)
        in_maps.append(
            {
                "feat0": feats_t[0][sl].reshape(128, -1),
                "feat1": feats_t[1][sl].reshape(128, -1),
                "feat2": feats_t[2][sl].reshape(128, -1),
                "boxes": boxes[sl],
                "c16": c16,
                "cP": cP,
            }
        )

    return run_bass_kernel_spmd(
        nc, in_maps, core_ids=list(range(N_CORES)), **run_kwargs
    )


def kernel(feat0, feat1, feat2, boxes):
    r = _run_device(feat0, feat1, feat2, boxes)
    total = np.float64(0.0)
    for m in r.results:
        total += np.float64(m["out"].reshape(-1)[0])

    count = B * N * len(PAIRS)
    avg = np.float32(total) / np.float32(count)
    loss = np.float32(1.0) - avg
    loss = np.nan_to_num(loss, nan=0.0, posinf=1.0, neginf=0.0)
    return np.array(np.clip(loss, 0.0, 2.0), dtype=np.float32)
